# revision 9
# baseline (speedup 1.0000x reference)
"""Trainium2 Bass kernel for nn_ReMMTAS (sparse stateful causal attention).

Strategy: 8-way tensor parallelism over the head axis (1 head per NeuronCore),
batch (B=2) kept on-core, segments sequential (state recurrence).

Math restructurings (all host-side, exact):
  - First attention's qkv: the input is head-broadcast, so StackedLinear
    collapses to a (256 -> dout) matmul with head-summed weights.
  - proj_inv + second qkv fused into one weight (Winv @ Wq2 per region),
    removing one head-mixing round-trip and one collective per segment.
  - RoPE feature pairs permuted (even/odd -> half-split) so the rotation is
    two partition-half blocks; permutation folded into q/k weight columns
    (scores are permutation-invariant).
  - 1/sqrt(dk) folded into q weights; W_out/H and W_state*0.5/H folded so
    per-head partial outputs simply sum across cores (host does the final sum
    for out_seg; a small fp32 AllReduce carries the recurrent state).

On-device dataflow per segment (per core, head g, per batch):
  fullT(256,768) -> qkv1 -> attn -> a1T(256,768) --AllGather--> (2048,768)
  -> fused qkv2 -> attn -> a2T --AllGather--> qkv3 -> attn -> a3T
  -> partial out_seg (fp32, host-summed) + partial state (fp32 AllReduce).

Everything on PE runs in fp16 (fp32 PSUM accumulation); softmax runs
unnormalized without max-subtraction (scores are small by construction),
with the denominator from an all-ones stationary matmul and applied via
DVE reciprocal+multiply after the PV matmul.
"""
import numpy as np

import concourse.bacc as bacc
import concourse.mybir as mybir
from concourse import tile
from concourse.bass_utils import run_bass_kernel_spmd

# ---- problem geometry (hardcoded per spec) ----
B, S, D = 2, 2048, 256
H = 8
SEG, ST = 512, 128
L = ST + SEG + ST          # 768
NSEG = S // SEG            # 4
DK, DV = 128, 256
NC = 8
KT1 = D // 128             # 2   (qkv1 contraction tiles)
KT = (H * D) // 128        # 16  (mixing contraction tiles)
LT = L // 128              # 6
REGIONS = ((0, ST), (ST, ST + SEG), (ST + SEG, L))

DT = mybir.dt.float16
DTF = mybir.dt.float32

_PERM = np.concatenate([np.arange(0, DK, 2), np.arange(1, DK, 2)])

_compiled = {}


# ======================================================================
# device program
# ======================================================================
def _build(nseg=NSEG):
    nc = bacc.Bacc("TRN2", target_bir_lowering=False, debug=False, num_devices=NC)

    xT = nc.dram_tensor("xT", [B, D, S], DT, kind="ExternalInput")
    istT = nc.dram_tensor("istT", [D, ST], DT, kind="ExternalInput")
    ropeC = nc.dram_tensor("ropeC", [NSEG, DK, L], DT, kind="ExternalInput")
    ropeS = nc.dram_tensor("ropeS", [NSEG, DK, L], DT, kind="ExternalInput")
    w1q = nc.dram_tensor("w1q", [3, 128, KT1 * 128], DT, kind="ExternalInput")
    w1k = nc.dram_tensor("w1k", [3, 128, KT1 * 128], DT, kind="ExternalInput")
    w1v = nc.dram_tensor("w1v", [3, 128, KT1 * 256], DT, kind="ExternalInput")
    w2q = nc.dram_tensor("w2q", [3, 128, KT * 128], DT, kind="ExternalInput")
    w2k = nc.dram_tensor("w2k", [3, 128, KT * 128], DT, kind="ExternalInput")
    w2v = nc.dram_tensor("w2v", [3, 128, KT * 256], DT, kind="ExternalInput")
    w3q = nc.dram_tensor("w3q", [3, 128, KT * 128], DT, kind="ExternalInput")
    w3k = nc.dram_tensor("w3k", [3, 128, KT * 128], DT, kind="ExternalInput")
    w3v = nc.dram_tensor("w3v", [3, 128, KT * 256], DT, kind="ExternalInput")
    wout = nc.dram_tensor("wout", [2, 128, 256], DT, kind="ExternalInput")
    wst = nc.dram_tensor("wst", [2, 128, 256], DT, kind="ExternalInput")
    mask01 = nc.dram_tensor("mask01", [128, 128], DT, kind="ExternalInput")

    ot = nc.dram_tensor("ot", [NSEG, B, D, SEG], DTF, kind="ExternalOutput")

    RG = [list(range(NC))]

    with tile.TileContext(nc) as tc:
        with (
            tc.tile_pool(name="dram", bufs=1, space="DRAM") as dp,
            tc.tile_pool(name="consts", bufs=1) as cp,
            tc.tile_pool(name="vw", bufs=3) as vwp,          # streamed v-weights
            tc.tile_pool(name="rope", bufs=2) as rp,
            tc.tile_pool(name="aall", bufs=18) as ap,        # gathered activations
            tc.tile_pool(name="work", bufs=4) as wp,         # qT/kT
            tc.tile_pool(name="vsb", bufs=12) as vp,
            tc.tile_pool(name="expp", bufs=12) as ep,
            tc.tile_pool(name="atp", bufs=4) as atp,
            tc.tile_pool(name="tmp", bufs=4) as tp,          # fp32 rope tmps
            tc.tile_pool(name="outp", bufs=4) as op_,
            tc.tile_pool(name="ps_qk", bufs=2, space="PSUM") as ps_qk,
            tc.tile_pool(name="ps_sv", bufs=2, space="PSUM") as ps_sv,
            tc.tile_pool(name="ps_acc", bufs=2, space="PSUM") as ps_acc,
        ):
            agin, agout, starin, starout = {}, {}, {}, {}
            for s in range(nseg):
                for m in (1, 2):
                    for b in range(B):
                        agin[s, m, b] = dp.tile([D, L], DT, tag=f"agin_{s}_{m}_{b}", name=f"agin_{s}_{m}_{b}")
                        agout[s, m, b] = dp.tile(
                            [H * D, L], DT, tag=f"agout_{s}_{m}_{b}",
                            addr_space="Shared", name=f"agout_{s}_{m}_{b}"
                        )
                starin[s] = dp.tile([B * D, ST], DTF, tag=f"starin_{s}", name=f"starin_{s}")
                starout[s] = dp.tile(
                    [B * D, ST], DTF, tag=f"starout_{s}",
                    addr_space="Shared", name=f"starout_{s}"
                )
            # ---------- resident constants ----------
            def load_const(name, dram, shape, sl=None):
                t = cp.tile(shape, DT, tag=name, name=name)
                nc.sync.dma_start(t[:], dram if sl is None else dram[sl])
                return t

            w1q_t = [load_const(f"w1q{r}", w1q[r], [128, KT1 * 128]) for r in range(3)]
            w1k_t = [load_const(f"w1k{r}", w1k[r], [128, KT1 * 128]) for r in range(3)]
            w1v_t = [load_const(f"w1v{r}", w1v[r], [128, KT1 * 256]) for r in range(3)]
            w2q_t = [load_const(f"w2q{r}", w2q[r], [128, KT * 128]) for r in range(3)]
            w2k_t = [load_const(f"w2k{r}", w2k[r], [128, KT * 128]) for r in range(3)]
            w3q_t = [load_const(f"w3q{r}", w3q[r], [128, KT * 128]) for r in range(3)]
            w3k_t = [load_const(f"w3k{r}", w3k[r], [128, KT * 128]) for r in range(3)]
            wout_t = [load_const(f"wout{k}", wout[k], [128, 256]) for k in range(2)]
            wst_t = [load_const(f"wst{k}", wst[k], [128, 256]) for k in range(2)]
            mask_t = load_const("mask", mask01[:], [128, 128])
            ones_t = cp.tile([128, 128], DT, tag="ones", name="ones_t")
            nc.vector.memset(ones_t[:], 1.0)

            # persistent state tiles (fp16), start from init state
            stT = {}
            for b in range(B):
                for kt in range(KT1):
                    t = cp.tile([128, ST], DT, tag=f"st{b}_{kt}", name=f"st{b}_{kt}")
                    nc.sync.dma_start(t[:], istT[kt * 128 : (kt + 1) * 128, :])
                    stT[b, kt] = t
            fullT = {
                (b, kt): cp.tile([128, L], DT, tag=f"full{b}_{kt}", name=f"full{b}_{kt}")
                for b in range(B)
                for kt in range(KT1)
            }

            # ---------- helpers ----------
            def chunks(lo, hi, step=512):
                # matmul outputs must not cross a PSUM bank boundary (512 fp32)
                out = []
                c = lo
                while c < hi:
                    bank_end = (c // 512 + 1) * 512
                    out.append((c, min(c + step, hi, bank_end)))
                    c = out[-1][1]
                return out

            def proj_qk(src, w_t, nkt, ropeC_t, ropeS_t, label):
                """q/k projection + rope: returns (128, L) fp16 tile."""
                ps = ps_qk.tile([DK, L], DTF, tag="qk", name="ps_qk_t")
                for r, (lo, hi) in enumerate(REGIONS):
                    for (c0, c1) in chunks(lo, hi):
                        for kt in range(nkt):
                            nc.tensor.matmul(
                                ps[:, c0:c1],
                                w_t[r][:, kt * 128 : (kt + 1) * 128],
                                src[kt][:, c0:c1],
                                start=(kt == 0),
                                stop=(kt == nkt - 1),
                            )
                # rope: out[0:64] = ps[0:64]*C - ps[64:]*S ; out[64:] = ps[64:]*C + ps[0:64]*S
                t = tp.tile([DK, L], DTF, tag="ropet", name="ropet")
                c = tp.tile([DK, L], DTF, tag="ropec", name="ropec")
                outT = wp.tile([DK, L], DT, tag=label, name=label)
                nc.vector.tensor_mul(t[0:64, :], ps[64:128, :], ropeS_t[0:64, :])
                nc.vector.tensor_mul(t[64:128, :], ps[0:64, :], ropeS_t[64:128, :])
                nc.vector.tensor_mul(c[:], ps[:], ropeC_t[:])
                nc.vector.tensor_sub(outT[0:64, :], c[0:64, :], t[0:64, :])
                nc.vector.tensor_add(outT[64:128, :], c[64:128, :], t[64:128, :])
                return outT

            def proj_v(src, w_t, nkt):
                """v projection in token-partition layout: 6 tiles (128, 256) fp16."""
                v_tiles = []
                for lt in range(LT):
                    r = 0 if lt == 0 else (2 if lt == LT - 1 else 1)
                    ps = ps_sv.tile([128, DV], DTF, tag="sv", name="ps_v_t")
                    for kt in range(nkt):
                        nc.tensor.matmul(
                            ps[:],
                            src[kt][:, lt * 128 : (lt + 1) * 128],
                            w_t[r][:, kt * 256 : (kt + 1) * 256],
                            start=(kt == 0),
                            stop=(kt == nkt - 1),
                        )
                    vt = vp.tile([128, DV], DT, tag="vsb", name="vsb_t")
                    nc.vector.tensor_copy(vt[:], ps[:])
                    v_tiles.append(vt)
                return v_tiles

            def attention(qT, kT, v_tiles):
                """causal attention; returns a^T as 2 tiles (128, L) fp16."""
                # scoresT blocks (j-partition, i-free), chunked at the 384 split
                expT = []
                for jt in range(LT):
                    et = ep.tile([128, L], DT, tag="exp", name="exp_t")
                    ccs = []
                    if jt * 128 < 384:
                        ccs.append((jt * 128, 384))
                    ccs.append((max(jt * 128, 384), L))
                    for (c0, c1) in ccs:
                        ps = ps_sv.tile([128, 384], DTF, tag="sv", name="ps_s_t")
                        nc.tensor.matmul(
                            ps[:, 0 : c1 - c0],
                            kT[:, jt * 128 : (jt + 1) * 128],
                            qT[:, c0:c1],
                            start=True,
                            stop=True,
                        )
                        nc.scalar.activation(
                            et[:, c0:c1], ps[:, 0 : c1 - c0],
                            mybir.ActivationFunctionType.Exp,
                        )
                    # causal mask on the diagonal block
                    nc.vector.tensor_mul(
                        et[:, jt * 128 : (jt + 1) * 128],
                        et[:, jt * 128 : (jt + 1) * 128],
                        mask_t[:],
                    )
                    expT.append(et)

                aT = [atp.tile([128, L], DT, tag="aT", name="aT_t") for _ in range(2)]
                for (i0, i1) in ((0, 384), (384, L)):
                    acc0 = ps_acc.tile([128, 384], DTF, tag="acc", name="acc0")
                    acc1 = ps_acc.tile([128, 384], DTF, tag="acc", name="acc1")
                    accs = ps_sv.tile([128, 384], DTF, tag="sv", name="accs")
                    jts = [jt for jt in range(LT) if jt * 128 < i1]
                    for n, jt in enumerate(jts):
                        c0 = max(jt * 128, i0)
                        first, last = (n == 0), (n == len(jts) - 1)
                        src = expT[jt][:, c0:i1]
                        dst = slice(c0 - i0, i1 - i0)
                        nc.tensor.matmul(acc0[:, dst], v_tiles[jt][:, 0:128], src,
                                         start=first, stop=last)
                        nc.tensor.matmul(acc1[:, dst], v_tiles[jt][:, 128:256], src,
                                         start=first, stop=last)
                        nc.tensor.matmul(accs[:, dst], ones_t[:], src,
                                         start=first, stop=last)
                    rec = tp.tile([128, 384], DTF, tag="rec", name="rec")
                    nc.vector.reciprocal_approx_fast(rec[:], accs[:])
                    nc.vector.tensor_mul(aT[0][:, i0:i1], acc0[:], rec[:])
                    nc.vector.tensor_mul(aT[1][:, i0:i1], acc1[:], rec[:])
                return aT

            # ---------- main sequence ----------
            for s in range(nseg):
                ropeC_t = rp.tile([DK, L], DT, tag="ropeC", name="ropeC_t")
                ropeS_t = rp.tile([DK, L], DT, tag="ropeS", name="ropeS_t")
                nc.sync.dma_start(ropeC_t[:], ropeC[s])
                nc.sync.dma_start(ropeS_t[:], ropeS[s])
                w2v_t = [vwp.tile([128, KT * 256], DT, tag="vw", name="w2v_t") for _ in range(3)]
                for r in range(3):
                    nc.sync.dma_start(w2v_t[r][:], w2v[r])

                a1T = {}
                for b in range(B):
                    # fullT = [stT | x_seg | stT]
                    for kt in range(KT1):
                        ft = fullT[b, kt]
                        nc.vector.tensor_copy(ft[:, 0:ST], stT[b, kt][:])
                        nc.sync.dma_start(
                            ft[:, ST : ST + SEG],
                            xT[b, kt * 128 : (kt + 1) * 128, s * SEG : (s + 1) * SEG],
                        )
                        nc.vector.tensor_copy(ft[:, ST + SEG : L], stT[b, kt][:])
                    src = [fullT[b, 0], fullT[b, 1]]
                    vt = proj_v(src, w1v_t, KT1)
                    qT = proj_qk(src, w1q_t, KT1, ropeC_t, ropeS_t, "qT")
                    kT = proj_qk(src, w1k_t, KT1, ropeC_t, ropeS_t, "kT")
                    a1T[b] = attention(qT, kT, vt)
                    for i in range(2):
                        nc.sync.dma_start(
                            agin[s, 1, b][i * 128 : (i + 1) * 128, :], a1T[b][i][:]
                        )
                    nc.gpsimd.collective_compute(
                        "AllGather", mybir.AluOpType.bypass, replica_groups=RG,
                        ins=[agin[s, 1, b].opt()], outs=[agout[s, 1, b].opt()],
                    )

                w3v_t = [vwp.tile([128, KT * 256], DT, tag="vw", name="w3v_t") for _ in range(3)]
                for r in range(3):
                    nc.sync.dma_start(w3v_t[r][:], w3v[r])

                a2T = {}
                for b in range(B):
                    aall = [ap.tile([128, L], DT, tag="aall", name="aall_t") for _ in range(KT)]
                    for kt in range(KT):
                        nc.sync.dma_start(
                            aall[kt][:], agout[s, 1, b][kt * 128 : (kt + 1) * 128, :]
                        )
                    vt = proj_v(aall, w2v_t, KT)
                    qT = proj_qk(aall, w2q_t, KT, ropeC_t, ropeS_t, "qT")
                    kT = proj_qk(aall, w2k_t, KT, ropeC_t, ropeS_t, "kT")
                    a2T[b] = attention(qT, kT, vt)
                    for i in range(2):
                        nc.sync.dma_start(
                            agin[s, 2, b][i * 128 : (i + 1) * 128, :], a2T[b][i][:]
                        )
                    nc.gpsimd.collective_compute(
                        "AllGather", mybir.AluOpType.bypass, replica_groups=RG,
                        ins=[agin[s, 2, b].opt()], outs=[agout[s, 2, b].opt()],
                    )

                for b in range(B):
                    aall = [ap.tile([128, L], DT, tag="aall", name="aall_t") for _ in range(KT)]
                    for kt in range(KT):
                        nc.sync.dma_start(
                            aall[kt][:], agout[s, 2, b][kt * 128 : (kt + 1) * 128, :]
                        )
                    vt = proj_v(aall, w3v_t, KT)
                    qT = proj_qk(aall, w3q_t, KT, ropeC_t, ropeS_t, "qT")
                    kT = proj_qk(aall, w3k_t, KT, ropeC_t, ropeS_t, "kT")
                    a3T = attention(qT, kT, vt)

                    # out partial: oT[h] = wout[:,h].T @ a3T[:, ST:ST+SEG]
                    for h in range(2):
                        ps = ps_qk.tile([128, SEG], DTF, tag="qk", name="ps_o")
                        for kt in range(2):
                            nc.tensor.matmul(
                                ps[:],
                                wout_t[kt][:, h * 128 : (h + 1) * 128],
                                a3T[kt][:, ST : ST + SEG],
                                start=(kt == 0),
                                stop=(kt == 1),
                            )
                        o32 = op_.tile([128, SEG], DTF, tag="o32", name="o32")
                        nc.vector.tensor_copy(o32[:], ps[:])
                        nc.sync.dma_start(ot[s, b, h * 128 : (h + 1) * 128, :], o32[:])

                    # state partial: stA = a3T[:, :ST] + a3T[:, -ST:]
                    stA = []
                    for kt in range(2):
                        t = op_.tile([128, ST], DT, tag="stA", name="stA_t")
                        nc.vector.tensor_add(
                            t[:], a3T[kt][:, 0:ST], a3T[kt][:, ST + SEG : L]
                        )
                        stA.append(t)
                    for h in range(2):
                        ps = ps_sv.tile([128, ST], DTF, tag="sv", name="ps_st")
                        for kt in range(2):
                            nc.tensor.matmul(
                                ps[:],
                                wst_t[kt][:, h * 128 : (h + 1) * 128],
                                stA[kt][:],
                                start=(kt == 0),
                                stop=(kt == 1),
                            )
                        s32 = op_.tile([128, ST], DTF, tag="s32", name="s32")
                        nc.vector.tensor_copy(s32[:], ps[:])
                        nc.sync.dma_start(
                            starin[s][b * D + h * 128 : b * D + (h + 1) * 128, :], s32[:]
                        )

                nc.gpsimd.collective_compute(
                    "AllReduce", mybir.AluOpType.add, replica_groups=RG,
                    ins=[starin[s].opt()], outs=[starout[s].opt()],
                )
                if s + 1 < nseg:
                    for b in range(B):
                        for kt in range(KT1):
                            t32 = tp.tile([128, ST], DTF, tag="st32", name="st32")
                            nc.sync.dma_start(
                                t32[:],
                                starout[s][b * D + kt * 128 : b * D + (kt + 1) * 128, :],
                            )
                            nc.vector.tensor_copy(stT[b, kt][:], t32[:])

    nc.compile()
    return nc


# ======================================================================
# host-side weight preparation
# ======================================================================
def _prep(x, params):
    lp0, lp1 = params["layers"]
    scale_q = 1.0 / np.sqrt(np.float32(DK))

    def regs(lp, nm):
        return [np.asarray(lp[nm + sfx], dtype=np.float32) for sfx in ("_s", "", "_e")]

    winv = [np.asarray(lp0[nm], dtype=np.float32) for nm in ("Winv_b", "Winv", "Winv_e")]

    # per-head weight blocks, laid out exactly as their SBUF tiles
    def pack_small(w_g, dout):
        # w_g: (256, dout) -> (128, KT1*dout): [p, kt*dout+c] = w[kt*128+p, c]
        return np.concatenate([w_g[kt * 128 : (kt + 1) * 128] for kt in range(KT1)], axis=1)

    def pack_big(w_g, dout):
        # w_g: (2048, dout) -> (128, KT*dout)
        return np.concatenate([w_g[kt * 128 : (kt + 1) * 128] for kt in range(KT)], axis=1)

    per_core = [dict() for _ in range(NC)]

    # qkv1: head-summed weights (input is head-broadcast)
    for nm, out_nm, perm, sc in (("Wq", "w1q", True, scale_q), ("Wk", "w1k", True, 1.0),
                                 ("Wv", "w1v", False, 1.0)):
        rr = regs(lp0, nm)
        for g in range(NC):
            blocks = []
            for r in range(3):
                w = rr[r].sum(axis=0)[g] * sc          # (256, dout)
                if perm:
                    w = w[:, _PERM]
                blocks.append(pack_small(w, w.shape[1]))
            per_core[g][out_nm] = np.stack(blocks).astype(np.float16)

    # fused proj_inv + qkv2
    for nm, out_nm, perm, sc in (("Wq", "w2q", True, scale_q), ("Wk", "w2k", True, 1.0),
                                 ("Wv", "w2v", False, 1.0)):
        rr = regs(lp0, nm)
        for r in range(3):
            # (h, f, d, o) = winv[h,g,d,e] @ wq[g,f,e,o]
            wf = np.einsum("hgde,gfeo->hfdo", winv[r], rr[r], optimize=True) * sc
            for g in range(NC):
                w = wf[:, g].reshape(H * D, -1)        # (2048, dout), h-major
                if perm:
                    w = w[:, _PERM]
                per_core[g].setdefault(out_nm, [None] * 3)[r] = pack_big(w, w.shape[1])
        for g in range(NC):
            per_core[g][out_nm] = np.stack(per_core[g][out_nm]).astype(np.float16)

    # layer-1 qkv
    for nm, out_nm, perm, sc in (("Wq", "w3q", True, scale_q), ("Wk", "w3k", True, 1.0),
                                 ("Wv", "w3v", False, 1.0)):
        rr = regs(lp1, nm)
        for g in range(NC):
            blocks = []
            for r in range(3):
                w = rr[r].transpose(1, 0, 2, 3)[g].reshape(H * D, -1) * sc
                if perm:
                    w = w[:, _PERM]
                blocks.append(pack_big(w, w.shape[1]))
            per_core[g][out_nm] = np.stack(blocks).astype(np.float16)

    wout = np.asarray(params["W_out"], dtype=np.float32) / H
    wst = np.asarray(params["W_state"], dtype=np.float32) * (0.5 / H)
    wout_p = np.stack([wout[k * 128 : (k + 1) * 128] for k in range(2)]).astype(np.float16)
    wst_p = np.stack([wst[k * 128 : (k + 1) * 128] for k in range(2)]).astype(np.float16)

    # rope tables (permuted order, duplicated halves)
    pos = (np.arange(NSEG)[:, None] * SEG + np.arange(L)[None, :]).astype(np.float64)
    inv = 1.0 / (10000.0 ** (np.arange(0, DK, 2) / DK))
    ang = pos[:, :, None] * inv[None, None, :]          # (NSEG, L, 64)
    cos = np.cos(ang).transpose(0, 2, 1)                # (NSEG, 64, L)
    sin = np.sin(ang).transpose(0, 2, 1)
    ropeC = np.concatenate([cos, cos], axis=1).astype(np.float16)
    ropeS = np.concatenate([sin, sin], axis=1).astype(np.float16)

    mask = (np.arange(128)[:, None] <= np.arange(128)[None, :]).astype(np.float16)

    xT = np.ascontiguousarray(np.asarray(x, dtype=np.float32).transpose(0, 2, 1)).astype(np.float16)
    istT = np.ascontiguousarray(
        np.asarray(params["init_state"], dtype=np.float32)[0].T
    ).astype(np.float16)

    in_maps = []
    for g in range(NC):
        m = dict(per_core[g])
        m["xT"] = xT
        m["istT"] = istT
        m["ropeC"] = ropeC
        m["ropeS"] = ropeS
        m["wout"] = wout_p
        m["wst"] = wst_p
        m["mask01"] = mask
        in_maps.append(m)
    return in_maps


def kernel(x, params):
    if "nc" not in _compiled:
        _compiled["nc"] = _build()
    nc = _compiled["nc"]
    in_maps = _prep(x, params)
    res = run_bass_kernel_spmd(nc, in_maps, list(range(NC)))
    out = np.zeros((B, S, D), dtype=np.float32)
    for c in range(NC):
        o = res.results[c]["ot"]                        # (NSEG, B, D, SEG) fp32
        for s in range(NSEG):
            for b in range(B):
                out[b, s * SEG : (s + 1) * SEG] += o[s, b].T
    return out


# revision 10
# speedup vs baseline: 1.0059x; 1.0059x over previous
"""Trainium2 Bass kernel for nn_ReMMTAS (sparse stateful causal attention).

Strategy: 8-way tensor parallelism over the head axis (1 head per NeuronCore),
batch (B=2) kept on-core, segments sequential (state recurrence).

Math restructurings (all host-side, exact):
  - First attention's qkv: the input is head-broadcast, so StackedLinear
    collapses to a (256 -> dout) matmul with head-summed weights.
  - proj_inv + second qkv fused into one weight (Winv @ Wq2 per region),
    removing one head-mixing round-trip and one collective per segment.
  - RoPE feature pairs permuted (even/odd -> half-split) so the rotation is
    two partition-half blocks; permutation folded into q/k weight columns
    (scores are permutation-invariant).
  - 1/sqrt(dk) folded into q weights; W_out/H and W_state*0.5/H folded so
    per-head partial outputs simply sum across cores (host does the final sum
    for out_seg; a small fp32 AllReduce carries the recurrent state).

On-device dataflow per segment (per core, head g, per batch):
  fullT(256,768) -> qkv1 -> attn -> a1T(256,768) --AllGather--> (2048,768)
  -> fused qkv2 -> attn -> a2T --AllGather--> qkv3 -> attn -> a3T
  -> partial out_seg (fp32, host-summed) + partial state (fp32 AllReduce).

Everything on PE runs in fp16 (fp32 PSUM accumulation); softmax runs
unnormalized without max-subtraction (scores are small by construction),
with the denominator from an all-ones stationary matmul and applied via
DVE reciprocal+multiply after the PV matmul.
"""
import numpy as np

import concourse.bacc as bacc
import concourse.mybir as mybir
from concourse import tile
from concourse.bass_utils import run_bass_kernel_spmd

# ---- problem geometry (hardcoded per spec) ----
B, S, D = 2, 2048, 256
H = 8
SEG, ST = 512, 128
L = ST + SEG + ST          # 768
NSEG = S // SEG            # 4
DK, DV = 128, 256
NC = 8
KT1 = D // 128             # 2   (qkv1 contraction tiles)
KT = (H * D) // 128        # 16  (mixing contraction tiles)
LT = L // 128              # 6
REGIONS = ((0, ST), (ST, ST + SEG), (ST + SEG, L))

DT = mybir.dt.float16
DTF = mybir.dt.float32

_PERM = np.concatenate([np.arange(0, DK, 2), np.arange(1, DK, 2)])

_compiled = {}


# ======================================================================
# device program
# ======================================================================
def _build(nseg=NSEG):
    nc = bacc.Bacc("TRN2", target_bir_lowering=False, debug=False, num_devices=NC)

    xT = nc.dram_tensor("xT", [B, D, S], DT, kind="ExternalInput")
    istT = nc.dram_tensor("istT", [D, ST], DT, kind="ExternalInput")
    ropeC = nc.dram_tensor("ropeC", [NSEG, DK, L], DT, kind="ExternalInput")
    ropeS = nc.dram_tensor("ropeS", [NSEG, DK, L], DT, kind="ExternalInput")
    w1q = nc.dram_tensor("w1q", [3, 128, KT1 * 128], DT, kind="ExternalInput")
    w1k = nc.dram_tensor("w1k", [3, 128, KT1 * 128], DT, kind="ExternalInput")
    w1v = nc.dram_tensor("w1v", [3, 128, KT1 * 256], DT, kind="ExternalInput")
    w2q = nc.dram_tensor("w2q", [3, 128, KT * 128], DT, kind="ExternalInput")
    w2k = nc.dram_tensor("w2k", [3, 128, KT * 128], DT, kind="ExternalInput")
    w2v = nc.dram_tensor("w2v", [3, 128, KT * 256], DT, kind="ExternalInput")
    w3q = nc.dram_tensor("w3q", [3, 128, KT * 128], DT, kind="ExternalInput")
    w3k = nc.dram_tensor("w3k", [3, 128, KT * 128], DT, kind="ExternalInput")
    w3v = nc.dram_tensor("w3v", [3, 128, KT * 256], DT, kind="ExternalInput")
    wout = nc.dram_tensor("wout", [2, 128, 256], DT, kind="ExternalInput")
    wst = nc.dram_tensor("wst", [2, 128, 256], DT, kind="ExternalInput")
    mask01 = nc.dram_tensor("mask01", [128, 128], DT, kind="ExternalInput")

    ot = nc.dram_tensor("ot", [NSEG, B, D, SEG], DTF, kind="ExternalOutput")

    RG = [list(range(NC))]

    with tile.TileContext(nc) as tc:
        with (
            tc.tile_pool(name="dram", bufs=1, space="DRAM") as dp,
            tc.tile_pool(name="consts", bufs=1) as cp,
            tc.tile_pool(name="vw", bufs=3) as vwp,          # streamed v-weights
            tc.tile_pool(name="rope", bufs=2) as rp,
            tc.tile_pool(name="aall", bufs=18) as ap,        # gathered activations
            tc.tile_pool(name="work", bufs=4) as wp,         # qT/kT
            tc.tile_pool(name="vsb", bufs=12) as vp,
            tc.tile_pool(name="expp", bufs=12) as ep,
            tc.tile_pool(name="atp", bufs=4) as atp,
            tc.tile_pool(name="tmp", bufs=4) as tp,          # fp32 rope tmps
            tc.tile_pool(name="outp", bufs=4) as op_,
            tc.tile_pool(name="ps_qk", bufs=2, space="PSUM") as ps_qk,
            tc.tile_pool(name="ps_sv", bufs=2, space="PSUM") as ps_sv,
            tc.tile_pool(name="ps_acc", bufs=2, space="PSUM") as ps_acc,
        ):
            agin, agout, starin, starout = {}, {}, {}, {}
            for s in range(nseg):
                for m in (1, 2):
                    for b in range(B):
                        agin[s, m, b] = dp.tile([D, L], DT, tag=f"agin_{s}_{m}_{b}", name=f"agin_{s}_{m}_{b}")
                        agout[s, m, b] = dp.tile(
                            [H * D, L], DT, tag=f"agout_{s}_{m}_{b}",
                            addr_space="Shared", name=f"agout_{s}_{m}_{b}"
                        )
                starin[s] = dp.tile([B * D, ST], DTF, tag=f"starin_{s}", name=f"starin_{s}")
                starout[s] = dp.tile(
                    [B * D, ST], DTF, tag=f"starout_{s}",
                    addr_space="Shared", name=f"starout_{s}"
                )
            # ---------- resident constants ----------
            def load_const(name, dram, shape, sl=None):
                t = cp.tile(shape, DT, tag=name, name=name)
                nc.sync.dma_start(t[:], dram if sl is None else dram[sl])
                return t

            w1q_t = [load_const(f"w1q{r}", w1q[r], [128, KT1 * 128]) for r in range(3)]
            w1k_t = [load_const(f"w1k{r}", w1k[r], [128, KT1 * 128]) for r in range(3)]
            w1v_t = [load_const(f"w1v{r}", w1v[r], [128, KT1 * 256]) for r in range(3)]
            w2q_t = [load_const(f"w2q{r}", w2q[r], [128, KT * 128]) for r in range(3)]
            w2k_t = [load_const(f"w2k{r}", w2k[r], [128, KT * 128]) for r in range(3)]
            w3q_t = [load_const(f"w3q{r}", w3q[r], [128, KT * 128]) for r in range(3)]
            w3k_t = [load_const(f"w3k{r}", w3k[r], [128, KT * 128]) for r in range(3)]
            wout_t = [load_const(f"wout{k}", wout[k], [128, 256]) for k in range(2)]
            wst_t = [load_const(f"wst{k}", wst[k], [128, 256]) for k in range(2)]
            mask_t = load_const("mask", mask01[:], [128, 128])
            ones_t = cp.tile([128, 128], DT, tag="ones", name="ones_t")
            nc.vector.memset(ones_t[:], 1.0)

            # per-region full tiles; S/E hold the recurrent state directly
            fullS, fullM, fullE = {}, {}, {}
            for b in range(B):
                for kt in range(KT1):
                    fullS[b, kt] = cp.tile([128, ST], DT, tag=f"fs{b}_{kt}", name=f"fs{b}_{kt}")
                    fullM[b, kt] = cp.tile([128, SEG], DT, tag=f"fm{b}_{kt}", name=f"fm{b}_{kt}")
                    fullE[b, kt] = cp.tile([128, ST], DT, tag=f"fe{b}_{kt}", name=f"fe{b}_{kt}")
                    nc.sync.dma_start(fullS[b, kt][:], istT[kt * 128 : (kt + 1) * 128, :])
                    nc.sync.dma_start(fullE[b, kt][:], istT[kt * 128 : (kt + 1) * 128, :])

            # ---------- helpers ----------
            def chunks(lo, hi, step=512):
                # matmul outputs must not cross a PSUM bank boundary (512 fp32)
                out = []
                c = lo
                while c < hi:
                    bank_end = (c // 512 + 1) * 512
                    out.append((c, min(c + step, hi, bank_end)))
                    c = out[-1][1]
                return out

            def proj_qk(src, w_t, nkt, ropeC_t, ropeS_t, label):
                """q/k projection + rope: returns (128, L) fp16 tile.
                src: either flat list of kt tiles (128, L), or (srcS, srcM, srcE)
                per-region kt tile lists."""
                per_region = isinstance(src, tuple)
                ps = ps_qk.tile([DK, L], DTF, tag="qk", name="ps_qk_t")
                for r, (lo, hi) in enumerate(REGIONS):
                    for (c0, c1) in chunks(lo, hi):
                        for kt in range(nkt):
                            mv = (src[r][kt][:, c0 - lo : c1 - lo] if per_region
                                  else src[kt][:, c0:c1])
                            nc.tensor.matmul(
                                ps[:, c0:c1],
                                w_t[r][:, kt * 128 : (kt + 1) * 128],
                                mv,
                                start=(kt == 0),
                                stop=(kt == nkt - 1),
                            )
                # rope: out[0:64] = ps[0:64]*C - ps[64:]*S ; out[64:] = ps[64:]*C + ps[0:64]*S
                t = tp.tile([DK, L], DTF, tag="ropet", name="ropet")
                c = tp.tile([DK, L], DTF, tag="ropec", name="ropec")
                outT = wp.tile([DK, L], DT, tag=label, name=label)
                nc.vector.tensor_mul(t[0:64, :], ps[64:128, :], ropeS_t[0:64, :])
                nc.vector.tensor_mul(t[64:128, :], ps[0:64, :], ropeS_t[64:128, :])
                nc.vector.tensor_mul(c[:], ps[:], ropeC_t[:])
                nc.vector.tensor_sub(outT[0:64, :], c[0:64, :], t[0:64, :])
                nc.vector.tensor_add(outT[64:128, :], c[64:128, :], t[64:128, :])
                return outT

            def proj_v(src, w_t, nkt):
                """v projection in token-partition layout: 6 tiles (128, 256) fp16."""
                per_region = isinstance(src, tuple)
                v_tiles = []
                for lt in range(LT):
                    r = 0 if lt == 0 else (2 if lt == LT - 1 else 1)
                    ps = ps_sv.tile([128, DV], DTF, tag="sv", name="ps_v_t")
                    for kt in range(nkt):
                        if per_region:
                            off = lt * 128 - REGIONS[r][0]
                            st_op = src[r][kt][:, off : off + 128]
                        else:
                            st_op = src[kt][:, lt * 128 : (lt + 1) * 128]
                        nc.tensor.matmul(
                            ps[:],
                            st_op,
                            w_t[r][:, kt * 256 : (kt + 1) * 256],
                            start=(kt == 0),
                            stop=(kt == nkt - 1),
                        )
                    vt = vp.tile([128, DV], DT, tag="vsb", name="vsb_t")
                    nc.vector.tensor_copy(vt[:], ps[:])
                    v_tiles.append(vt)
                return v_tiles

            def attention(qT, kT, v_tiles):
                """causal attention; returns a^T as 2 tiles (128, L) fp16."""
                # scoresT blocks (j-partition, i-free), chunked at the 384 split
                expT = []
                for jt in range(LT):
                    et = ep.tile([128, L], DT, tag="exp", name="exp_t")
                    ccs = []
                    if jt * 128 < 384:
                        ccs.append((jt * 128, 384))
                    ccs.append((max(jt * 128, 384), L))
                    for (c0, c1) in ccs:
                        ps = ps_sv.tile([128, 384], DTF, tag="sv", name="ps_s_t")
                        nc.tensor.matmul(
                            ps[:, 0 : c1 - c0],
                            kT[:, jt * 128 : (jt + 1) * 128],
                            qT[:, c0:c1],
                            start=True,
                            stop=True,
                        )
                        nc.scalar.activation(
                            et[:, c0:c1], ps[:, 0 : c1 - c0],
                            mybir.ActivationFunctionType.Exp,
                        )
                    # causal mask on the diagonal block
                    nc.vector.tensor_mul(
                        et[:, jt * 128 : (jt + 1) * 128],
                        et[:, jt * 128 : (jt + 1) * 128],
                        mask_t[:],
                    )
                    expT.append(et)

                aT = [atp.tile([128, L], DT, tag="aT", name="aT_t") for _ in range(2)]
                for (i0, i1) in ((0, 384), (384, L)):
                    acc0 = ps_acc.tile([128, 384], DTF, tag="acc", name="acc0")
                    acc1 = ps_acc.tile([128, 384], DTF, tag="acc", name="acc1")
                    accs = ps_sv.tile([128, 384], DTF, tag="sv", name="accs")
                    jts = [jt for jt in range(LT) if jt * 128 < i1]
                    for n, jt in enumerate(jts):
                        c0 = max(jt * 128, i0)
                        first, last = (n == 0), (n == len(jts) - 1)
                        src = expT[jt][:, c0:i1]
                        dst = slice(c0 - i0, i1 - i0)
                        nc.tensor.matmul(acc0[:, dst], v_tiles[jt][:, 0:128], src,
                                         start=first, stop=last)
                        nc.tensor.matmul(acc1[:, dst], v_tiles[jt][:, 128:256], src,
                                         start=first, stop=last)
                        nc.tensor.matmul(accs[:, dst], ones_t[:], src,
                                         start=first, stop=last)
                    rec = tp.tile([128, 384], DTF, tag="rec", name="rec")
                    nc.vector.reciprocal_approx_fast(rec[:], accs[:])
                    nc.vector.tensor_mul(aT[0][:, i0:i1], acc0[:], rec[:])
                    nc.vector.tensor_mul(aT[1][:, i0:i1], acc1[:], rec[:])
                return aT

            # ---------- main sequence ----------
            for s in range(nseg):
                ropeC_t = rp.tile([DK, L], DT, tag="ropeC", name="ropeC_t")
                ropeS_t = rp.tile([DK, L], DT, tag="ropeS", name="ropeS_t")
                nc.sync.dma_start(ropeC_t[:], ropeC[s])
                nc.sync.dma_start(ropeS_t[:], ropeS[s])
                w2v_t = [vwp.tile([128, KT * 256], DT, tag="vw", name="w2v_t") for _ in range(3)]
                for r in range(3):
                    nc.sync.dma_start(w2v_t[r][:], w2v[r])

                a1T = {}
                for b in range(B):
                    for kt in range(KT1):
                        nc.sync.dma_start(
                            fullM[b, kt][:],
                            xT[b, kt * 128 : (kt + 1) * 128, s * SEG : (s + 1) * SEG],
                        )
                    src = (
                        [fullS[b, 0], fullS[b, 1]],
                        [fullM[b, 0], fullM[b, 1]],
                        [fullE[b, 0], fullE[b, 1]],
                    )
                    vt = proj_v(src, w1v_t, KT1)
                    qT = proj_qk(src, w1q_t, KT1, ropeC_t, ropeS_t, "qT")
                    kT = proj_qk(src, w1k_t, KT1, ropeC_t, ropeS_t, "kT")
                    a1T[b] = attention(qT, kT, vt)
                    for i in range(2):
                        nc.sync.dma_start(
                            agin[s, 1, b][i * 128 : (i + 1) * 128, :], a1T[b][i][:]
                        )
                    nc.gpsimd.collective_compute(
                        "AllGather", mybir.AluOpType.bypass, replica_groups=RG,
                        ins=[agin[s, 1, b].opt()], outs=[agout[s, 1, b].opt()],
                    )

                w3v_t = [vwp.tile([128, KT * 256], DT, tag="vw", name="w3v_t") for _ in range(3)]
                for r in range(3):
                    nc.sync.dma_start(w3v_t[r][:], w3v[r])

                a2T = {}
                for b in range(B):
                    aall = [ap.tile([128, L], DT, tag="aall", name="aall_t") for _ in range(KT)]
                    for kt in range(KT):
                        nc.sync.dma_start(
                            aall[kt][:], agout[s, 1, b][kt * 128 : (kt + 1) * 128, :]
                        )
                    vt = proj_v(aall, w2v_t, KT)
                    qT = proj_qk(aall, w2q_t, KT, ropeC_t, ropeS_t, "qT")
                    kT = proj_qk(aall, w2k_t, KT, ropeC_t, ropeS_t, "kT")
                    a2T[b] = attention(qT, kT, vt)
                    for i in range(2):
                        nc.sync.dma_start(
                            agin[s, 2, b][i * 128 : (i + 1) * 128, :], a2T[b][i][:]
                        )
                    nc.gpsimd.collective_compute(
                        "AllGather", mybir.AluOpType.bypass, replica_groups=RG,
                        ins=[agin[s, 2, b].opt()], outs=[agout[s, 2, b].opt()],
                    )

                for b in range(B):
                    aall = [ap.tile([128, L], DT, tag="aall", name="aall_t") for _ in range(KT)]
                    for kt in range(KT):
                        nc.sync.dma_start(
                            aall[kt][:], agout[s, 2, b][kt * 128 : (kt + 1) * 128, :]
                        )
                    vt = proj_v(aall, w3v_t, KT)
                    qT = proj_qk(aall, w3q_t, KT, ropeC_t, ropeS_t, "qT")
                    kT = proj_qk(aall, w3k_t, KT, ropeC_t, ropeS_t, "kT")
                    a3T = attention(qT, kT, vt)

                    # state partial first (feeds the AllReduce on the critical path)
                    stA = []
                    for kt in range(2):
                        t = op_.tile([128, ST], DT, tag="stA", name="stA_t")
                        nc.vector.tensor_add(
                            t[:], a3T[kt][:, 0:ST], a3T[kt][:, ST + SEG : L]
                        )
                        stA.append(t)
                    for h in range(2):
                        ps = ps_sv.tile([128, ST], DTF, tag="sv", name="ps_st")
                        for kt in range(2):
                            nc.tensor.matmul(
                                ps[:],
                                wst_t[kt][:, h * 128 : (h + 1) * 128],
                                stA[kt][:],
                                start=(kt == 0),
                                stop=(kt == 1),
                            )
                        s32 = op_.tile([128, ST], DTF, tag="s32", name="s32")
                        nc.vector.tensor_copy(s32[:], ps[:])
                        nc.sync.dma_start(
                            starin[s][b * D + h * 128 : b * D + (h + 1) * 128, :], s32[:]
                        )

                    # out partial: oT[h] = wout[:,h].T @ a3T[:, ST:ST+SEG]
                    for h in range(2):
                        ps = ps_qk.tile([128, SEG], DTF, tag="qk", name="ps_o")
                        for kt in range(2):
                            nc.tensor.matmul(
                                ps[:],
                                wout_t[kt][:, h * 128 : (h + 1) * 128],
                                a3T[kt][:, ST : ST + SEG],
                                start=(kt == 0),
                                stop=(kt == 1),
                            )
                        o32 = op_.tile([128, SEG], DTF, tag="o32", name="o32")
                        nc.vector.tensor_copy(o32[:], ps[:])
                        nc.sync.dma_start(ot[s, b, h * 128 : (h + 1) * 128, :], o32[:])

                nc.gpsimd.collective_compute(
                    "AllReduce", mybir.AluOpType.add, replica_groups=RG,
                    ins=[starin[s].opt()], outs=[starout[s].opt()],
                )
                if s + 1 < nseg:
                    for b in range(B):
                        for kt in range(KT1):
                            t32 = tp.tile([128, ST], DTF, tag="st32", name="st32")
                            nc.sync.dma_start(
                                t32[:],
                                starout[s][b * D + kt * 128 : b * D + (kt + 1) * 128, :],
                            )
                            nc.vector.tensor_copy(fullS[b, kt][:], t32[:])
                            nc.vector.tensor_copy(fullE[b, kt][:], t32[:])

    nc.compile()
    return nc


# ======================================================================
# host-side weight preparation
# ======================================================================
def _prep(x, params):
    lp0, lp1 = params["layers"]
    scale_q = 1.0 / np.sqrt(np.float32(DK))

    def regs(lp, nm):
        return [np.asarray(lp[nm + sfx], dtype=np.float32) for sfx in ("_s", "", "_e")]

    winv = [np.asarray(lp0[nm], dtype=np.float32) for nm in ("Winv_b", "Winv", "Winv_e")]

    # per-head weight blocks, laid out exactly as their SBUF tiles
    def pack_small(w_g, dout):
        # w_g: (256, dout) -> (128, KT1*dout): [p, kt*dout+c] = w[kt*128+p, c]
        return np.concatenate([w_g[kt * 128 : (kt + 1) * 128] for kt in range(KT1)], axis=1)

    def pack_big(w_g, dout):
        # w_g: (2048, dout) -> (128, KT*dout)
        return np.concatenate([w_g[kt * 128 : (kt + 1) * 128] for kt in range(KT)], axis=1)

    per_core = [dict() for _ in range(NC)]

    # qkv1: head-summed weights (input is head-broadcast)
    for nm, out_nm, perm, sc in (("Wq", "w1q", True, scale_q), ("Wk", "w1k", True, 1.0),
                                 ("Wv", "w1v", False, 1.0)):
        rr = regs(lp0, nm)
        for g in range(NC):
            blocks = []
            for r in range(3):
                w = rr[r].sum(axis=0)[g] * sc          # (256, dout)
                if perm:
                    w = w[:, _PERM]
                blocks.append(pack_small(w, w.shape[1]))
            per_core[g][out_nm] = np.stack(blocks).astype(np.float16)

    # fused proj_inv + qkv2
    for nm, out_nm, perm, sc in (("Wq", "w2q", True, scale_q), ("Wk", "w2k", True, 1.0),
                                 ("Wv", "w2v", False, 1.0)):
        rr = regs(lp0, nm)
        for r in range(3):
            # (h, f, d, o) = winv[h,g,d,e] @ wq[g,f,e,o]
            wf = np.einsum("hgde,gfeo->hfdo", winv[r], rr[r], optimize=True) * sc
            for g in range(NC):
                w = wf[:, g].reshape(H * D, -1)        # (2048, dout), h-major
                if perm:
                    w = w[:, _PERM]
                per_core[g].setdefault(out_nm, [None] * 3)[r] = pack_big(w, w.shape[1])
        for g in range(NC):
            per_core[g][out_nm] = np.stack(per_core[g][out_nm]).astype(np.float16)

    # layer-1 qkv
    for nm, out_nm, perm, sc in (("Wq", "w3q", True, scale_q), ("Wk", "w3k", True, 1.0),
                                 ("Wv", "w3v", False, 1.0)):
        rr = regs(lp1, nm)
        for g in range(NC):
            blocks = []
            for r in range(3):
                w = rr[r].transpose(1, 0, 2, 3)[g].reshape(H * D, -1) * sc
                if perm:
                    w = w[:, _PERM]
                blocks.append(pack_big(w, w.shape[1]))
            per_core[g][out_nm] = np.stack(blocks).astype(np.float16)

    wout = np.asarray(params["W_out"], dtype=np.float32) / H
    wst = np.asarray(params["W_state"], dtype=np.float32) * (0.5 / H)
    wout_p = np.stack([wout[k * 128 : (k + 1) * 128] for k in range(2)]).astype(np.float16)
    wst_p = np.stack([wst[k * 128 : (k + 1) * 128] for k in range(2)]).astype(np.float16)

    # rope tables (permuted order, duplicated halves)
    pos = (np.arange(NSEG)[:, None] * SEG + np.arange(L)[None, :]).astype(np.float64)
    inv = 1.0 / (10000.0 ** (np.arange(0, DK, 2) / DK))
    ang = pos[:, :, None] * inv[None, None, :]          # (NSEG, L, 64)
    cos = np.cos(ang).transpose(0, 2, 1)                # (NSEG, 64, L)
    sin = np.sin(ang).transpose(0, 2, 1)
    ropeC = np.concatenate([cos, cos], axis=1).astype(np.float16)
    ropeS = np.concatenate([sin, sin], axis=1).astype(np.float16)

    mask = (np.arange(128)[:, None] <= np.arange(128)[None, :]).astype(np.float16)

    xT = np.ascontiguousarray(np.asarray(x, dtype=np.float32).transpose(0, 2, 1)).astype(np.float16)
    istT = np.ascontiguousarray(
        np.asarray(params["init_state"], dtype=np.float32)[0].T
    ).astype(np.float16)

    in_maps = []
    for g in range(NC):
        m = dict(per_core[g])
        m["xT"] = xT
        m["istT"] = istT
        m["ropeC"] = ropeC
        m["ropeS"] = ropeS
        m["wout"] = wout_p
        m["wst"] = wst_p
        m["mask01"] = mask
        in_maps.append(m)
    return in_maps


def kernel(x, params):
    if "nc" not in _compiled:
        _compiled["nc"] = _build()
    nc = _compiled["nc"]
    in_maps = _prep(x, params)
    res = run_bass_kernel_spmd(nc, in_maps, list(range(NC)))
    out = np.zeros((B, S, D), dtype=np.float32)
    for c in range(NC):
        o = res.results[c]["ot"]                        # (NSEG, B, D, SEG) fp32
        for s in range(NSEG):
            for b in range(B):
                out[b, s * SEG : (s + 1) * SEG] += o[s, b].T
    return out


# revision 11
# speedup vs baseline: 1.0229x; 1.0169x over previous
"""Trainium2 Bass kernel for nn_ReMMTAS (sparse stateful causal attention).

Strategy: 8-way tensor parallelism over the head axis (1 head per NeuronCore),
batch (B=2) kept on-core, segments sequential (state recurrence).

Math restructurings (all host-side, exact):
  - First attention's qkv: the input is head-broadcast, so StackedLinear
    collapses to a (256 -> dout) matmul with head-summed weights.
  - proj_inv + second qkv fused into one weight (Winv @ Wq2 per region),
    removing one head-mixing round-trip and one collective per segment.
  - RoPE feature pairs permuted (even/odd -> half-split) so the rotation is
    two partition-half blocks; permutation folded into q/k weight columns
    (scores are permutation-invariant).
  - 1/sqrt(dk) folded into q weights; W_out/H and W_state*0.5/H folded so
    per-head partial outputs simply sum across cores (host does the final sum
    for out_seg; a small fp32 AllReduce carries the recurrent state).

On-device dataflow per segment (per core, head g, per batch):
  fullT(256,768) -> qkv1 -> attn -> a1T(256,768) --AllGather--> (2048,768)
  -> fused qkv2 -> attn -> a2T --AllGather--> qkv3 -> attn -> a3T
  -> partial out_seg (fp32, host-summed) + partial state (fp32 AllReduce).

Everything on PE runs in fp16 (fp32 PSUM accumulation); softmax runs
unnormalized without max-subtraction (scores are small by construction),
with the denominator from an all-ones stationary matmul and applied via
DVE reciprocal+multiply after the PV matmul.
"""
import numpy as np

import concourse.bacc as bacc
import concourse.mybir as mybir
from concourse import tile
from concourse.bass_utils import run_bass_kernel_spmd

# ---- problem geometry (hardcoded per spec) ----
B, S, D = 2, 2048, 256
H = 8
SEG, ST = 512, 128
L = ST + SEG + ST          # 768
NSEG = S // SEG            # 4
DK, DV = 128, 256
NC = 8
KT1 = D // 128             # 2   (qkv1 contraction tiles)
KT = (H * D) // 128        # 16  (mixing contraction tiles)
LT = L // 128              # 6
REGIONS = ((0, ST), (ST, ST + SEG), (ST + SEG, L))

DT = mybir.dt.float16
DTF = mybir.dt.float32

_PERM = np.concatenate([np.arange(0, DK, 2), np.arange(1, DK, 2)])

_compiled = {}


# ======================================================================
# device program
# ======================================================================
def _build(nseg=NSEG):
    nc = bacc.Bacc("TRN2", target_bir_lowering=False, debug=False, num_devices=NC)

    xT = nc.dram_tensor("xT", [B, D, S], DT, kind="ExternalInput")
    istT = nc.dram_tensor("istT", [D, ST], DT, kind="ExternalInput")
    ropeC = nc.dram_tensor("ropeC", [NSEG, DK, L], DT, kind="ExternalInput")
    ropeS = nc.dram_tensor("ropeS", [NSEG, DK, L], DT, kind="ExternalInput")
    w1q = nc.dram_tensor("w1q", [3, 128, KT1 * 128], DT, kind="ExternalInput")
    w1k = nc.dram_tensor("w1k", [3, 128, KT1 * 128], DT, kind="ExternalInput")
    w1v = nc.dram_tensor("w1v", [3, 128, KT1 * 256], DT, kind="ExternalInput")
    w2q = nc.dram_tensor("w2q", [3, 128, KT * 128], DT, kind="ExternalInput")
    w2k = nc.dram_tensor("w2k", [3, 128, KT * 128], DT, kind="ExternalInput")
    w2v = nc.dram_tensor("w2v", [3, 128, KT * 256], DT, kind="ExternalInput")
    w3q = nc.dram_tensor("w3q", [3, 128, KT * 128], DT, kind="ExternalInput")
    w3k = nc.dram_tensor("w3k", [3, 128, KT * 128], DT, kind="ExternalInput")
    w3v = nc.dram_tensor("w3v", [3, 128, KT * 256], DT, kind="ExternalInput")
    wout = nc.dram_tensor("wout", [2, 128, 256], DT, kind="ExternalInput")
    wst = nc.dram_tensor("wst", [2, 128, 256], DT, kind="ExternalInput")
    mask01 = nc.dram_tensor("mask01", [128, 128], DT, kind="ExternalInput")

    ot = nc.dram_tensor("ot", [NSEG, B, D, SEG], DTF, kind="ExternalOutput")

    RG = [list(range(NC))]

    with tile.TileContext(nc) as tc:
        with (
            tc.tile_pool(name="dram", bufs=1, space="DRAM") as dp,
            tc.tile_pool(name="consts", bufs=1) as cp,
            tc.tile_pool(name="vw", bufs=3) as vwp,          # streamed v-weights
            tc.tile_pool(name="rope", bufs=2) as rp,
            tc.tile_pool(name="aall", bufs=18) as ap,        # gathered activations
            tc.tile_pool(name="work", bufs=4) as wp,         # qT/kT
            tc.tile_pool(name="vsb", bufs=12) as vp,
            tc.tile_pool(name="expp", bufs=12) as ep,
            tc.tile_pool(name="atp", bufs=4) as atp,
            tc.tile_pool(name="tmp", bufs=4) as tp,          # fp32 rope tmps
            tc.tile_pool(name="outp", bufs=4) as op_,
            tc.tile_pool(name="ps_qk", bufs=2, space="PSUM") as ps_qk,
            tc.tile_pool(name="ps_sv", bufs=2, space="PSUM") as ps_sv,
            tc.tile_pool(name="ps_acc", bufs=2, space="PSUM") as ps_acc,
        ):
            agin, agout, starin, starout = {}, {}, {}, {}
            for s in range(nseg):
                for m in (1, 2):
                    for b in range(B):
                        agin[s, m, b] = dp.tile([D, L], DT, tag=f"agin_{s}_{m}_{b}", name=f"agin_{s}_{m}_{b}")
                        agout[s, m, b] = dp.tile(
                            [H * D, L], DT, tag=f"agout_{s}_{m}_{b}",
                            addr_space="Shared", name=f"agout_{s}_{m}_{b}"
                        )
                starin[s] = dp.tile([B * D, ST], DTF, tag=f"starin_{s}", name=f"starin_{s}")
                starout[s] = dp.tile(
                    [B * D, ST], DTF, tag=f"starout_{s}",
                    addr_space="Shared", name=f"starout_{s}"
                )
            # ---------- resident constants ----------
            def load_const(name, dram, shape, sl=None):
                t = cp.tile(shape, DT, tag=name, name=name)
                nc.sync.dma_start(t[:], dram if sl is None else dram[sl])
                return t

            w1q_t = [load_const(f"w1q{r}", w1q[r], [128, KT1 * 128]) for r in range(3)]
            w1k_t = [load_const(f"w1k{r}", w1k[r], [128, KT1 * 128]) for r in range(3)]
            w1v_t = [load_const(f"w1v{r}", w1v[r], [128, KT1 * 256]) for r in range(3)]
            w2q_t = [load_const(f"w2q{r}", w2q[r], [128, KT * 128]) for r in range(3)]
            w2k_t = [load_const(f"w2k{r}", w2k[r], [128, KT * 128]) for r in range(3)]
            w3q_t = [load_const(f"w3q{r}", w3q[r], [128, KT * 128]) for r in range(3)]
            w3k_t = [load_const(f"w3k{r}", w3k[r], [128, KT * 128]) for r in range(3)]
            wout_t = [load_const(f"wout{k}", wout[k], [128, 256]) for k in range(2)]
            wst_t = [load_const(f"wst{k}", wst[k], [128, 256]) for k in range(2)]
            mask_t = load_const("mask", mask01[:], [128, 128])
            ones_t = cp.tile([128, 128], DT, tag="ones", name="ones_t")
            nc.vector.memset(ones_t[:], 1.0)

            # per-region full tiles; S/E hold the recurrent state directly
            fullS, fullM, fullE = {}, {}, {}
            for b in range(B):
                for kt in range(KT1):
                    fullS[b, kt] = cp.tile([128, ST], DT, tag=f"fs{b}_{kt}", name=f"fs{b}_{kt}")
                    fullM[b, kt] = cp.tile([128, SEG], DT, tag=f"fm{b}_{kt}", name=f"fm{b}_{kt}")
                    fullE[b, kt] = cp.tile([128, ST], DT, tag=f"fe{b}_{kt}", name=f"fe{b}_{kt}")
                    nc.sync.dma_start(fullS[b, kt][:], istT[kt * 128 : (kt + 1) * 128, :])
                    nc.sync.dma_start(fullE[b, kt][:], istT[kt * 128 : (kt + 1) * 128, :])

            # ---------- helpers ----------
            def chunks(lo, hi, step=512):
                # matmul outputs must not cross a PSUM bank boundary (512 fp32)
                out = []
                c = lo
                while c < hi:
                    bank_end = (c // 512 + 1) * 512
                    out.append((c, min(c + step, hi, bank_end)))
                    c = out[-1][1]
                return out

            def proj_qk(src, w_t, nkt, ropeC_t, ropeS_t, label):
                """q/k projection + rope: returns (128, L) fp16 tile.
                src: either flat list of kt tiles (128, L), or (srcS, srcM, srcE)
                per-region kt tile lists."""
                per_region = isinstance(src, tuple)
                ps = ps_qk.tile([DK, L], DTF, tag="qk", name="ps_qk_t")
                for r in (1, 0, 2):
                    lo, hi = REGIONS[r]
                    for (c0, c1) in chunks(lo, hi):
                        for kt in range(nkt):
                            mv = (src[r][kt][:, c0 - lo : c1 - lo] if per_region
                                  else src[kt][:, c0:c1])
                            nc.tensor.matmul(
                                ps[:, c0:c1],
                                w_t[r][:, kt * 128 : (kt + 1) * 128],
                                mv,
                                start=(kt == 0),
                                stop=(kt == nkt - 1),
                            )
                # rope: out[0:64] = ps[0:64]*C - ps[64:]*S ; out[64:] = ps[64:]*C + ps[0:64]*S
                t = tp.tile([DK, L], DTF, tag="ropet", name="ropet")
                c = tp.tile([DK, L], DTF, tag="ropec", name="ropec")
                outT = wp.tile([DK, L], DT, tag=label, name=label)
                nc.vector.tensor_mul(t[0:64, :], ps[64:128, :], ropeS_t[0:64, :])
                nc.vector.tensor_mul(t[64:128, :], ps[0:64, :], ropeS_t[64:128, :])
                nc.vector.tensor_mul(c[:], ps[:], ropeC_t[:])
                nc.vector.tensor_sub(outT[0:64, :], c[0:64, :], t[0:64, :])
                nc.vector.tensor_add(outT[64:128, :], c[64:128, :], t[64:128, :])
                return outT

            def proj_v(src, w_t, nkt):
                """v projection in token-partition layout: 6 tiles (128, 256) fp16."""
                per_region = isinstance(src, tuple)
                v_tiles = [None] * LT
                for lt in (1, 2, 3, 4, 0, 5):
                    r = 0 if lt == 0 else (2 if lt == LT - 1 else 1)
                    ps = ps_sv.tile([128, DV], DTF, tag="sv", name="ps_v_t")
                    for kt in range(nkt):
                        if per_region:
                            off = lt * 128 - REGIONS[r][0]
                            st_op = src[r][kt][:, off : off + 128]
                        else:
                            st_op = src[kt][:, lt * 128 : (lt + 1) * 128]
                        nc.tensor.matmul(
                            ps[:],
                            st_op,
                            w_t[r][:, kt * 256 : (kt + 1) * 256],
                            start=(kt == 0),
                            stop=(kt == nkt - 1),
                        )
                    vt = vp.tile([128, DV], DT, tag="vsb", name="vsb_t")
                    nc.vector.tensor_copy(vt[:], ps[:])
                    v_tiles[lt] = vt
                return v_tiles

            def attention(qT, kT, v_tiles):
                """causal attention; returns a^T as 2 tiles (128, L) fp16."""
                # scoresT blocks (j-partition, i-free), chunked at the 384 split
                expT = []
                for jt in range(LT):
                    et = ep.tile([128, L], DT, tag="exp", name="exp_t")
                    ccs = []
                    if jt * 128 < 384:
                        ccs.append((jt * 128, 384))
                    ccs.append((max(jt * 128, 384), L))
                    for (c0, c1) in ccs:
                        ps = ps_sv.tile([128, 384], DTF, tag="sv", name="ps_s_t")
                        nc.tensor.matmul(
                            ps[:, 0 : c1 - c0],
                            kT[:, jt * 128 : (jt + 1) * 128],
                            qT[:, c0:c1],
                            start=True,
                            stop=True,
                        )
                        nc.scalar.activation(
                            et[:, c0:c1], ps[:, 0 : c1 - c0],
                            mybir.ActivationFunctionType.Exp,
                        )
                    # causal mask on the diagonal block
                    nc.vector.tensor_mul(
                        et[:, jt * 128 : (jt + 1) * 128],
                        et[:, jt * 128 : (jt + 1) * 128],
                        mask_t[:],
                    )
                    expT.append(et)

                aT = [atp.tile([128, L], DT, tag="aT", name="aT_t") for _ in range(2)]
                for (i0, i1) in ((0, 384), (384, L)):
                    acc0 = ps_acc.tile([128, 384], DTF, tag="acc", name="acc0")
                    acc1 = ps_acc.tile([128, 384], DTF, tag="acc", name="acc1")
                    accs = ps_sv.tile([128, 384], DTF, tag="sv", name="accs")
                    jts = [jt for jt in range(LT) if jt * 128 < i1]
                    for n, jt in enumerate(jts):
                        c0 = max(jt * 128, i0)
                        first, last = (n == 0), (n == len(jts) - 1)
                        src = expT[jt][:, c0:i1]
                        dst = slice(c0 - i0, i1 - i0)
                        nc.tensor.matmul(acc0[:, dst], v_tiles[jt][:, 0:128], src,
                                         start=first, stop=last)
                        nc.tensor.matmul(acc1[:, dst], v_tiles[jt][:, 128:256], src,
                                         start=first, stop=last)
                        nc.tensor.matmul(accs[:, dst], ones_t[:], src,
                                         start=first, stop=last)
                    rec = tp.tile([128, 384], DTF, tag="rec", name="rec")
                    nc.vector.reciprocal_approx_fast(rec[:], accs[:])
                    nc.vector.tensor_mul(aT[0][:, i0:i1], acc0[:], rec[:])
                    nc.vector.tensor_mul(aT[1][:, i0:i1], acc1[:], rec[:])
                return aT

            # ---------- main sequence ----------
            for s in range(nseg):
                ropeC_t = rp.tile([DK, L], DT, tag="ropeC", name="ropeC_t")
                ropeS_t = rp.tile([DK, L], DT, tag="ropeS", name="ropeS_t")
                nc.sync.dma_start(ropeC_t[:], ropeC[s])
                nc.sync.dma_start(ropeS_t[:], ropeS[s])
                w2v_t = [vwp.tile([128, KT * 256], DT, tag="vw", name="w2v_t") for _ in range(3)]
                for r in range(3):
                    nc.sync.dma_start(w2v_t[r][:], w2v[r])

                a1T = {}
                for b in range(B):
                    for kt in range(KT1):
                        nc.sync.dma_start(
                            fullM[b, kt][:],
                            xT[b, kt * 128 : (kt + 1) * 128, s * SEG : (s + 1) * SEG],
                        )
                    src = (
                        [fullS[b, 0], fullS[b, 1]],
                        [fullM[b, 0], fullM[b, 1]],
                        [fullE[b, 0], fullE[b, 1]],
                    )
                    vt = proj_v(src, w1v_t, KT1)
                    qT = proj_qk(src, w1q_t, KT1, ropeC_t, ropeS_t, "qT")
                    kT = proj_qk(src, w1k_t, KT1, ropeC_t, ropeS_t, "kT")
                    a1T[b] = attention(qT, kT, vt)
                    for i in range(2):
                        for (c0, c1) in ((0, 384), (384, L)):
                            nc.sync.dma_start(
                                agin[s, 1, b][i * 128 : (i + 1) * 128, c0:c1],
                                a1T[b][i][:, c0:c1],
                            )
                    nc.gpsimd.collective_compute(
                        "AllGather", mybir.AluOpType.bypass, replica_groups=RG,
                        ins=[agin[s, 1, b].opt()], outs=[agout[s, 1, b].opt()],
                    )

                w3v_t = [vwp.tile([128, KT * 256], DT, tag="vw", name="w3v_t") for _ in range(3)]
                for r in range(3):
                    nc.sync.dma_start(w3v_t[r][:], w3v[r])

                a2T = {}
                for b in range(B):
                    aall = [ap.tile([128, L], DT, tag="aall", name="aall_t") for _ in range(KT)]
                    for kt in range(KT):
                        nc.sync.dma_start(
                            aall[kt][:], agout[s, 1, b][kt * 128 : (kt + 1) * 128, :]
                        )
                    vt = proj_v(aall, w2v_t, KT)
                    qT = proj_qk(aall, w2q_t, KT, ropeC_t, ropeS_t, "qT")
                    kT = proj_qk(aall, w2k_t, KT, ropeC_t, ropeS_t, "kT")
                    a2T[b] = attention(qT, kT, vt)
                    for i in range(2):
                        for (c0, c1) in ((0, 384), (384, L)):
                            nc.sync.dma_start(
                                agin[s, 2, b][i * 128 : (i + 1) * 128, c0:c1],
                                a2T[b][i][:, c0:c1],
                            )
                    nc.gpsimd.collective_compute(
                        "AllGather", mybir.AluOpType.bypass, replica_groups=RG,
                        ins=[agin[s, 2, b].opt()], outs=[agout[s, 2, b].opt()],
                    )

                for b in range(B):
                    aall = [ap.tile([128, L], DT, tag="aall", name="aall_t") for _ in range(KT)]
                    for kt in range(KT):
                        nc.sync.dma_start(
                            aall[kt][:], agout[s, 2, b][kt * 128 : (kt + 1) * 128, :]
                        )
                    vt = proj_v(aall, w3v_t, KT)
                    qT = proj_qk(aall, w3q_t, KT, ropeC_t, ropeS_t, "qT")
                    kT = proj_qk(aall, w3k_t, KT, ropeC_t, ropeS_t, "kT")
                    a3T = attention(qT, kT, vt)

                    # state partial first (feeds the AllReduce on the critical path)
                    stA = []
                    for kt in range(2):
                        t = op_.tile([128, ST], DT, tag="stA", name="stA_t")
                        nc.vector.tensor_add(
                            t[:], a3T[kt][:, 0:ST], a3T[kt][:, ST + SEG : L]
                        )
                        stA.append(t)
                    for h in range(2):
                        ps = ps_sv.tile([128, ST], DTF, tag="sv", name="ps_st")
                        for kt in range(2):
                            nc.tensor.matmul(
                                ps[:],
                                wst_t[kt][:, h * 128 : (h + 1) * 128],
                                stA[kt][:],
                                start=(kt == 0),
                                stop=(kt == 1),
                            )
                        s32 = op_.tile([128, ST], DTF, tag="s32", name="s32")
                        nc.vector.tensor_copy(s32[:], ps[:])
                        nc.sync.dma_start(
                            starin[s][b * D + h * 128 : b * D + (h + 1) * 128, :], s32[:]
                        )

                    # out partial: oT[h] = wout[:,h].T @ a3T[:, ST:ST+SEG]
                    for h in range(2):
                        ps = ps_qk.tile([128, SEG], DTF, tag="qk", name="ps_o")
                        for kt in range(2):
                            nc.tensor.matmul(
                                ps[:],
                                wout_t[kt][:, h * 128 : (h + 1) * 128],
                                a3T[kt][:, ST : ST + SEG],
                                start=(kt == 0),
                                stop=(kt == 1),
                            )
                        o32 = op_.tile([128, SEG], DTF, tag="o32", name="o32")
                        nc.vector.tensor_copy(o32[:], ps[:])
                        nc.sync.dma_start(ot[s, b, h * 128 : (h + 1) * 128, :], o32[:])

                nc.gpsimd.collective_compute(
                    "AllReduce", mybir.AluOpType.add, replica_groups=RG,
                    ins=[starin[s].opt()], outs=[starout[s].opt()],
                )
                if s + 1 < nseg:
                    for b in range(B):
                        for kt in range(KT1):
                            t32 = tp.tile([128, ST], DTF, tag="st32", name="st32")
                            nc.sync.dma_start(
                                t32[:],
                                starout[s][b * D + kt * 128 : b * D + (kt + 1) * 128, :],
                            )
                            nc.vector.tensor_copy(fullS[b, kt][:], t32[:])
                            nc.vector.tensor_copy(fullE[b, kt][:], t32[:])

    nc.compile()
    return nc


# ======================================================================
# host-side weight preparation
# ======================================================================
def _prep(x, params):
    lp0, lp1 = params["layers"]
    scale_q = 1.0 / np.sqrt(np.float32(DK))

    def regs(lp, nm):
        return [np.asarray(lp[nm + sfx], dtype=np.float32) for sfx in ("_s", "", "_e")]

    winv = [np.asarray(lp0[nm], dtype=np.float32) for nm in ("Winv_b", "Winv", "Winv_e")]

    # per-head weight blocks, laid out exactly as their SBUF tiles
    def pack_small(w_g, dout):
        # w_g: (256, dout) -> (128, KT1*dout): [p, kt*dout+c] = w[kt*128+p, c]
        return np.concatenate([w_g[kt * 128 : (kt + 1) * 128] for kt in range(KT1)], axis=1)

    def pack_big(w_g, dout):
        # w_g: (2048, dout) -> (128, KT*dout)
        return np.concatenate([w_g[kt * 128 : (kt + 1) * 128] for kt in range(KT)], axis=1)

    per_core = [dict() for _ in range(NC)]

    # qkv1: head-summed weights (input is head-broadcast)
    for nm, out_nm, perm, sc in (("Wq", "w1q", True, scale_q), ("Wk", "w1k", True, 1.0),
                                 ("Wv", "w1v", False, 1.0)):
        rr = regs(lp0, nm)
        for g in range(NC):
            blocks = []
            for r in range(3):
                w = rr[r].sum(axis=0)[g] * sc          # (256, dout)
                if perm:
                    w = w[:, _PERM]
                blocks.append(pack_small(w, w.shape[1]))
            per_core[g][out_nm] = np.stack(blocks).astype(np.float16)

    # fused proj_inv + qkv2
    for nm, out_nm, perm, sc in (("Wq", "w2q", True, scale_q), ("Wk", "w2k", True, 1.0),
                                 ("Wv", "w2v", False, 1.0)):
        rr = regs(lp0, nm)
        for r in range(3):
            # (h, f, d, o) = winv[h,g,d,e] @ wq[g,f,e,o]
            wf = np.einsum("hgde,gfeo->hfdo", winv[r], rr[r], optimize=True) * sc
            for g in range(NC):
                w = wf[:, g].reshape(H * D, -1)        # (2048, dout), h-major
                if perm:
                    w = w[:, _PERM]
                per_core[g].setdefault(out_nm, [None] * 3)[r] = pack_big(w, w.shape[1])
        for g in range(NC):
            per_core[g][out_nm] = np.stack(per_core[g][out_nm]).astype(np.float16)

    # layer-1 qkv
    for nm, out_nm, perm, sc in (("Wq", "w3q", True, scale_q), ("Wk", "w3k", True, 1.0),
                                 ("Wv", "w3v", False, 1.0)):
        rr = regs(lp1, nm)
        for g in range(NC):
            blocks = []
            for r in range(3):
                w = rr[r].transpose(1, 0, 2, 3)[g].reshape(H * D, -1) * sc
                if perm:
                    w = w[:, _PERM]
                blocks.append(pack_big(w, w.shape[1]))
            per_core[g][out_nm] = np.stack(blocks).astype(np.float16)

    wout = np.asarray(params["W_out"], dtype=np.float32) / H
    wst = np.asarray(params["W_state"], dtype=np.float32) * (0.5 / H)
    wout_p = np.stack([wout[k * 128 : (k + 1) * 128] for k in range(2)]).astype(np.float16)
    wst_p = np.stack([wst[k * 128 : (k + 1) * 128] for k in range(2)]).astype(np.float16)

    # rope tables (permuted order, duplicated halves)
    pos = (np.arange(NSEG)[:, None] * SEG + np.arange(L)[None, :]).astype(np.float64)
    inv = 1.0 / (10000.0 ** (np.arange(0, DK, 2) / DK))
    ang = pos[:, :, None] * inv[None, None, :]          # (NSEG, L, 64)
    cos = np.cos(ang).transpose(0, 2, 1)                # (NSEG, 64, L)
    sin = np.sin(ang).transpose(0, 2, 1)
    ropeC = np.concatenate([cos, cos], axis=1).astype(np.float16)
    ropeS = np.concatenate([sin, sin], axis=1).astype(np.float16)

    mask = (np.arange(128)[:, None] <= np.arange(128)[None, :]).astype(np.float16)

    xT = np.ascontiguousarray(np.asarray(x, dtype=np.float32).transpose(0, 2, 1)).astype(np.float16)
    istT = np.ascontiguousarray(
        np.asarray(params["init_state"], dtype=np.float32)[0].T
    ).astype(np.float16)

    in_maps = []
    for g in range(NC):
        m = dict(per_core[g])
        m["xT"] = xT
        m["istT"] = istT
        m["ropeC"] = ropeC
        m["ropeS"] = ropeS
        m["wout"] = wout_p
        m["wst"] = wst_p
        m["mask01"] = mask
        in_maps.append(m)
    return in_maps


def kernel(x, params):
    if "nc" not in _compiled:
        _compiled["nc"] = _build()
    nc = _compiled["nc"]
    in_maps = _prep(x, params)
    res = run_bass_kernel_spmd(nc, in_maps, list(range(NC)))
    out = np.zeros((B, S, D), dtype=np.float32)
    for c in range(NC):
        o = res.results[c]["ot"]                        # (NSEG, B, D, SEG) fp32
        for s in range(NSEG):
            for b in range(B):
                out[b, s * SEG : (s + 1) * SEG] += o[s, b].T
    return out


# revision 13
# speedup vs baseline: 1.0371x; 1.0138x over previous
"""Trainium2 Bass kernel for nn_ReMMTAS (sparse stateful causal attention).

Strategy: 8-way tensor parallelism over the head axis (1 head per NeuronCore),
batch (B=2) kept on-core, segments sequential (state recurrence).

Math restructurings (all host-side, exact):
  - First attention's qkv: the input is head-broadcast, so StackedLinear
    collapses to a (256 -> dout) matmul with head-summed weights.
  - proj_inv + second qkv fused into one weight (Winv @ Wq2 per region),
    removing one head-mixing round-trip and one collective per segment.
  - RoPE feature pairs permuted (even/odd -> half-split) so the rotation is
    two partition-half blocks; permutation folded into q/k weight columns
    (scores are permutation-invariant).
  - 1/sqrt(dk) folded into q weights; W_out/H and W_state*0.5/H folded so
    per-head partial outputs simply sum across cores (host does the final sum
    for out_seg; a small fp32 AllReduce carries the recurrent state).

On-device dataflow per segment (per core, head g, per batch):
  fullT(256,768) -> qkv1 -> attn -> a1T(256,768) --AllGather--> (2048,768)
  -> fused qkv2 -> attn -> a2T --AllGather--> qkv3 -> attn -> a3T
  -> partial out_seg (fp32, host-summed) + partial state (fp32 AllReduce).

Everything on PE runs in fp16 (fp32 PSUM accumulation); softmax runs
unnormalized without max-subtraction (scores are small by construction),
with the denominator from an all-ones stationary matmul and applied via
DVE reciprocal+multiply after the PV matmul.
"""
import numpy as np

import concourse.bacc as bacc
import concourse.mybir as mybir
from concourse import tile
from concourse.bass_utils import run_bass_kernel_spmd

# ---- problem geometry (hardcoded per spec) ----
B, S, D = 2, 2048, 256
H = 8
SEG, ST = 512, 128
L = ST + SEG + ST          # 768
NSEG = S // SEG            # 4
DK, DV = 128, 256
NC = 8
KT1 = D // 128             # 2   (qkv1 contraction tiles)
KT = (H * D) // 128        # 16  (mixing contraction tiles)
LT = L // 128              # 6
REGIONS = ((0, ST), (ST, ST + SEG), (ST + SEG, L))

DT = mybir.dt.float16
DTF = mybir.dt.float32

_PERM = np.concatenate([np.arange(0, DK, 2), np.arange(1, DK, 2)])

_compiled = {}


# ======================================================================
# device program
# ======================================================================
def _build(nseg=NSEG):
    nc = bacc.Bacc("TRN2", target_bir_lowering=False, debug=False, num_devices=NC)

    xT = nc.dram_tensor("xT", [B, D, S], DT, kind="ExternalInput")
    istT = nc.dram_tensor("istT", [D, ST], DT, kind="ExternalInput")
    ropeC = nc.dram_tensor("ropeC", [NSEG, DK, L], DT, kind="ExternalInput")
    ropeS = nc.dram_tensor("ropeS", [NSEG, DK, L], DT, kind="ExternalInput")
    w1q = nc.dram_tensor("w1q", [3, 128, KT1 * 128], DT, kind="ExternalInput")
    w1k = nc.dram_tensor("w1k", [3, 128, KT1 * 128], DT, kind="ExternalInput")
    w1v = nc.dram_tensor("w1v", [3, 128, KT1 * 256], DT, kind="ExternalInput")
    w2q = nc.dram_tensor("w2q", [3, 128, KT * 128], DT, kind="ExternalInput")
    w2k = nc.dram_tensor("w2k", [3, 128, KT * 128], DT, kind="ExternalInput")
    w2v = nc.dram_tensor("w2v", [3, 128, KT * 256], DT, kind="ExternalInput")
    w3q = nc.dram_tensor("w3q", [3, 128, KT * 128], DT, kind="ExternalInput")
    w3k = nc.dram_tensor("w3k", [3, 128, KT * 128], DT, kind="ExternalInput")
    w3v = nc.dram_tensor("w3v", [3, 128, KT * 256], DT, kind="ExternalInput")
    wout = nc.dram_tensor("wout", [2, 128, 256], DT, kind="ExternalInput")
    wst = nc.dram_tensor("wst", [2, 128, 256], DT, kind="ExternalInput")
    mask01 = nc.dram_tensor("mask01", [128, 128], DT, kind="ExternalInput")

    ot = nc.dram_tensor("ot", [NSEG, B, D, SEG], DTF, kind="ExternalOutput")

    RG = [list(range(NC))]

    with tile.TileContext(nc) as tc:
        with (
            tc.tile_pool(name="dram", bufs=1, space="DRAM") as dp,
            tc.tile_pool(name="consts", bufs=1) as cp,
            tc.tile_pool(name="vw", bufs=3) as vwp,          # streamed v-weights
            tc.tile_pool(name="rope", bufs=2) as rp,
            tc.tile_pool(name="aall", bufs=18) as ap,        # gathered activations
            tc.tile_pool(name="work", bufs=4) as wp,         # qT/kT
            tc.tile_pool(name="vsb", bufs=12) as vp,
            tc.tile_pool(name="expp", bufs=12) as ep,
            tc.tile_pool(name="atp", bufs=4) as atp,
            tc.tile_pool(name="tmp", bufs=4) as tp,          # fp32 rope tmps
            tc.tile_pool(name="outp", bufs=2) as op_,
            tc.tile_pool(name="ps_qk", bufs=2, space="PSUM") as ps_qk,
            tc.tile_pool(name="ps_sv", bufs=2, space="PSUM") as ps_sv,
            tc.tile_pool(name="ps_acc", bufs=2, space="PSUM") as ps_acc,
        ):
            agin, agout, starin, starout = {}, {}, {}, {}
            for s in range(nseg):
                for m in (1, 2):
                    for b in range(B):
                        agin[s, m, b] = dp.tile([D, L], DT, tag=f"agin_{s}_{m}_{b}", name=f"agin_{s}_{m}_{b}")
                        agout[s, m, b] = dp.tile(
                            [H * D, L], DT, tag=f"agout_{s}_{m}_{b}",
                            addr_space="Shared", name=f"agout_{s}_{m}_{b}"
                        )
                starin[s] = dp.tile([B * D, ST], DTF, tag=f"starin_{s}", name=f"starin_{s}")
                starout[s] = dp.tile(
                    [B * D, ST], DTF, tag=f"starout_{s}",
                    addr_space="Shared", name=f"starout_{s}"
                )
            # ---------- resident constants ----------
            def load_const(name, dram, shape, sl=None):
                t = cp.tile(shape, DT, tag=name, name=name)
                nc.sync.dma_start(t[:], dram if sl is None else dram[sl])
                return t

            w1q_t = [load_const(f"w1q{r}", w1q[r], [128, KT1 * 128]) for r in range(3)]
            w1k_t = [load_const(f"w1k{r}", w1k[r], [128, KT1 * 128]) for r in range(3)]
            w1v_t = [load_const(f"w1v{r}", w1v[r], [128, KT1 * 256]) for r in range(3)]
            w2q_t = [load_const(f"w2q{r}", w2q[r], [128, KT * 128]) for r in range(3)]
            w2k_t = [load_const(f"w2k{r}", w2k[r], [128, KT * 128]) for r in range(3)]
            w3q_t = [load_const(f"w3q{r}", w3q[r], [128, KT * 128]) for r in range(3)]
            w3k_t = [load_const(f"w3k{r}", w3k[r], [128, KT * 128]) for r in range(3)]
            wout_t = [load_const(f"wout{k}", wout[k], [128, 256]) for k in range(2)]
            wst_t = [load_const(f"wst{k}", wst[k], [128, 256]) for k in range(2)]
            mask_t = load_const("mask", mask01[:], [128, 128])
            ones_t = cp.tile([128, 128], DT, tag="ones", name="ones_t")
            nc.vector.memset(ones_t[:], 1.0)

            # per-region full tiles; S/E hold the recurrent state directly.
            # M is double-buffered by segment parity and prefetched a segment
            # early so its DMA never queues behind the state AllReduce.
            fullS, fullM, fullE = {}, {}, {}
            for b in range(B):
                for kt in range(KT1):
                    fullS[b, kt] = cp.tile([128, ST], DT, tag=f"fs{b}_{kt}", name=f"fs{b}_{kt}")
                    fullE[b, kt] = cp.tile([128, ST], DT, tag=f"fe{b}_{kt}", name=f"fe{b}_{kt}")
                    for p in range(2):
                        fullM[b, kt, p] = cp.tile(
                            [128, SEG], DT, tag=f"fm{b}_{kt}{p}", name=f"fm{b}_{kt}{p}"
                        )
                    nc.sync.dma_start(fullS[b, kt][:], istT[kt * 128 : (kt + 1) * 128, :])
                    nc.sync.dma_start(fullE[b, kt][:], istT[kt * 128 : (kt + 1) * 128, :])

            def load_fullM(s):
                for b in range(B):
                    for kt in range(KT1):
                        nc.sync.dma_start(
                            fullM[b, kt, s % 2][:],
                            xT[b, kt * 128 : (kt + 1) * 128, s * SEG : (s + 1) * SEG],
                        )

            # ---------- helpers ----------
            def chunks(lo, hi, step=512):
                # matmul outputs must not cross a PSUM bank boundary (512 fp32)
                out = []
                c = lo
                while c < hi:
                    bank_end = (c // 512 + 1) * 512
                    out.append((c, min(c + step, hi, bank_end)))
                    c = out[-1][1]
                return out

            def proj_qk(src, w_t, nkt, ropeC_t, ropeS_t, label):
                """q/k projection + rope: returns (128, L) fp16 tile.
                src: either flat list of kt tiles (128, L), or (srcS, srcM, srcE)
                per-region kt tile lists."""
                per_region = isinstance(src, tuple)
                ps = ps_qk.tile([DK, L], DTF, tag="qk", name="ps_qk_t")
                for r in (1, 0, 2):
                    lo, hi = REGIONS[r]
                    for (c0, c1) in chunks(lo, hi):
                        for kt in range(nkt):
                            mv = (src[r][kt][:, c0 - lo : c1 - lo] if per_region
                                  else src[kt][:, c0:c1])
                            nc.tensor.matmul(
                                ps[:, c0:c1],
                                w_t[r][:, kt * 128 : (kt + 1) * 128],
                                mv,
                                start=(kt == 0),
                                stop=(kt == nkt - 1),
                            )
                # rope: out[0:64] = ps[0:64]*C - ps[64:]*S ; out[64:] = ps[64:]*C + ps[0:64]*S
                t = tp.tile([DK, L], DTF, tag="ropet", name="ropet")
                c = tp.tile([DK, L], DTF, tag="ropec", name="ropec")
                outT = wp.tile([DK, L], DT, tag=label, name=label)
                nc.vector.tensor_mul(t[0:64, :], ps[64:128, :], ropeS_t[0:64, :])
                nc.vector.tensor_mul(t[64:128, :], ps[0:64, :], ropeS_t[64:128, :])
                nc.vector.tensor_mul(c[:], ps[:], ropeC_t[:])
                nc.vector.tensor_sub(outT[0:64, :], c[0:64, :], t[0:64, :])
                nc.vector.tensor_add(outT[64:128, :], c[64:128, :], t[64:128, :])
                return outT

            def proj_v(src, w_t, nkt):
                """v projection in token-partition layout: 6 tiles (128, 256) fp16."""
                per_region = isinstance(src, tuple)
                v_tiles = [None] * LT
                for lt in (1, 2, 3, 4, 0, 5):
                    r = 0 if lt == 0 else (2 if lt == LT - 1 else 1)
                    ps = ps_sv.tile([128, DV], DTF, tag="sv", name="ps_v_t")
                    for kt in range(nkt):
                        if per_region:
                            off = lt * 128 - REGIONS[r][0]
                            st_op = src[r][kt][:, off : off + 128]
                        else:
                            st_op = src[kt][:, lt * 128 : (lt + 1) * 128]
                        nc.tensor.matmul(
                            ps[:],
                            st_op,
                            w_t[r][:, kt * 256 : (kt + 1) * 256],
                            start=(kt == 0),
                            stop=(kt == nkt - 1),
                        )
                    vt = vp.tile([128, DV], DT, tag="vsb", name="vsb_t")
                    nc.vector.tensor_copy(vt[:], ps[:])
                    v_tiles[lt] = vt
                return v_tiles

            def attention(qT, kT, v_tiles):
                """causal attention; returns a^T as 2 tiles (128, L) fp16."""
                # scoresT blocks (j-partition, i-free), chunked at the 384 split
                expT = []
                for jt in range(LT):
                    et = ep.tile([128, L], DT, tag="exp", name="exp_t")
                    ccs = []
                    if jt * 128 < 384:
                        ccs.append((jt * 128, 384))
                    ccs.append((max(jt * 128, 384), L))
                    for (c0, c1) in ccs:
                        ps = ps_sv.tile([128, 384], DTF, tag="sv", name="ps_s_t")
                        nc.tensor.matmul(
                            ps[:, 0 : c1 - c0],
                            kT[:, jt * 128 : (jt + 1) * 128],
                            qT[:, c0:c1],
                            start=True,
                            stop=True,
                        )
                        nc.scalar.activation(
                            et[:, c0:c1], ps[:, 0 : c1 - c0],
                            mybir.ActivationFunctionType.Exp,
                        )
                    # causal mask on the diagonal block
                    nc.vector.tensor_mul(
                        et[:, jt * 128 : (jt + 1) * 128],
                        et[:, jt * 128 : (jt + 1) * 128],
                        mask_t[:],
                    )
                    expT.append(et)

                aT = [atp.tile([128, L], DT, tag="aT", name="aT_t") for _ in range(2)]
                for (i0, i1) in ((0, 384), (384, L)):
                    acc0 = ps_acc.tile([128, 384], DTF, tag="acc", name="acc0")
                    acc1 = ps_acc.tile([128, 384], DTF, tag="acc", name="acc1")
                    accs = ps_sv.tile([128, 384], DTF, tag="sv", name="accs")
                    jts = [jt for jt in range(LT) if jt * 128 < i1]
                    for n, jt in enumerate(jts):
                        c0 = max(jt * 128, i0)
                        first, last = (n == 0), (n == len(jts) - 1)
                        src = expT[jt][:, c0:i1]
                        dst = slice(c0 - i0, i1 - i0)
                        nc.tensor.matmul(acc0[:, dst], v_tiles[jt][:, 0:128], src,
                                         start=first, stop=last)
                        nc.tensor.matmul(acc1[:, dst], v_tiles[jt][:, 128:256], src,
                                         start=first, stop=last)
                        nc.tensor.matmul(accs[:, dst], ones_t[:], src,
                                         start=first, stop=last)
                    rec = tp.tile([128, 384], DTF, tag="rec", name="rec")
                    nc.vector.reciprocal_approx_fast(rec[:], accs[:])
                    nc.vector.tensor_mul(aT[0][:, i0:i1], acc0[:], rec[:])
                    nc.vector.tensor_mul(aT[1][:, i0:i1], acc1[:], rec[:])
                return aT

            # ---------- main sequence ----------
            for s in range(nseg):
                ropeC_t = rp.tile([DK, L], DT, tag="ropeC", name="ropeC_t")
                ropeS_t = rp.tile([DK, L], DT, tag="ropeS", name="ropeS_t")
                nc.sync.dma_start(ropeC_t[:], ropeC[s])
                nc.sync.dma_start(ropeS_t[:], ropeS[s])
                if s == 0:
                    load_fullM(0)
                if s + 1 < nseg:
                    load_fullM(s + 1)
                w2v_t = [vwp.tile([128, KT * 256], DT, tag="vw", name="w2v_t") for _ in range(3)]
                for r in range(3):
                    nc.sync.dma_start(w2v_t[r][:], w2v[r])

                a1T = {}
                for b in range(B):
                    src = (
                        [fullS[b, 0], fullS[b, 1]],
                        [fullM[b, 0, s % 2], fullM[b, 1, s % 2]],
                        [fullE[b, 0], fullE[b, 1]],
                    )
                    vt = proj_v(src, w1v_t, KT1)
                    qT = proj_qk(src, w1q_t, KT1, ropeC_t, ropeS_t, "qT")
                    kT = proj_qk(src, w1k_t, KT1, ropeC_t, ropeS_t, "kT")
                    a1T[b] = attention(qT, kT, vt)
                    for i in range(2):
                        for (c0, c1) in ((0, 384), (384, L)):
                            nc.sync.dma_start(
                                agin[s, 1, b][i * 128 : (i + 1) * 128, c0:c1],
                                a1T[b][i][:, c0:c1],
                            )
                    nc.gpsimd.collective_compute(
                        "AllGather", mybir.AluOpType.bypass, replica_groups=RG,
                        ins=[agin[s, 1, b].opt()], outs=[agout[s, 1, b].opt()],
                    )

                w3v_t = [vwp.tile([128, KT * 256], DT, tag="vw", name="w3v_t") for _ in range(3)]
                for r in range(3):
                    nc.sync.dma_start(w3v_t[r][:], w3v[r])

                a2T = {}
                for b in range(B):
                    aall = [ap.tile([128, L], DT, tag="aall", name="aall_t") for _ in range(KT)]
                    for kt in range(KT):
                        nc.sync.dma_start(
                            aall[kt][:], agout[s, 1, b][kt * 128 : (kt + 1) * 128, :]
                        )
                    vt = proj_v(aall, w2v_t, KT)
                    qT = proj_qk(aall, w2q_t, KT, ropeC_t, ropeS_t, "qT")
                    kT = proj_qk(aall, w2k_t, KT, ropeC_t, ropeS_t, "kT")
                    a2T[b] = attention(qT, kT, vt)
                    for i in range(2):
                        for (c0, c1) in ((0, 384), (384, L)):
                            nc.sync.dma_start(
                                agin[s, 2, b][i * 128 : (i + 1) * 128, c0:c1],
                                a2T[b][i][:, c0:c1],
                            )
                    nc.gpsimd.collective_compute(
                        "AllGather", mybir.AluOpType.bypass, replica_groups=RG,
                        ins=[agin[s, 2, b].opt()], outs=[agout[s, 2, b].opt()],
                    )

                for b in range(B):
                    aall = [ap.tile([128, L], DT, tag="aall", name="aall_t") for _ in range(KT)]
                    for kt in range(KT):
                        nc.sync.dma_start(
                            aall[kt][:], agout[s, 2, b][kt * 128 : (kt + 1) * 128, :]
                        )
                    vt = proj_v(aall, w3v_t, KT)
                    qT = proj_qk(aall, w3q_t, KT, ropeC_t, ropeS_t, "qT")
                    kT = proj_qk(aall, w3k_t, KT, ropeC_t, ropeS_t, "kT")
                    a3T = attention(qT, kT, vt)

                    # state partial first (feeds the AllReduce on the critical path)
                    stA = []
                    for kt in range(2):
                        t = op_.tile([128, ST], DT, tag="stA", name="stA_t")
                        nc.vector.tensor_add(
                            t[:], a3T[kt][:, 0:ST], a3T[kt][:, ST + SEG : L]
                        )
                        stA.append(t)
                    for h in range(2):
                        ps = ps_sv.tile([128, ST], DTF, tag="sv", name="ps_st")
                        for kt in range(2):
                            nc.tensor.matmul(
                                ps[:],
                                wst_t[kt][:, h * 128 : (h + 1) * 128],
                                stA[kt][:],
                                start=(kt == 0),
                                stop=(kt == 1),
                            )
                        s32 = op_.tile([128, ST], DTF, tag="s32", name="s32")
                        nc.vector.tensor_copy(s32[:], ps[:])
                        nc.sync.dma_start(
                            starin[s][b * D + h * 128 : b * D + (h + 1) * 128, :], s32[:]
                        )

                    # out partial: oT[h] = wout[:,h].T @ a3T[:, ST:ST+SEG]
                    for h in range(2):
                        ps = ps_qk.tile([128, SEG], DTF, tag="qk", name="ps_o")
                        for kt in range(2):
                            nc.tensor.matmul(
                                ps[:],
                                wout_t[kt][:, h * 128 : (h + 1) * 128],
                                a3T[kt][:, ST : ST + SEG],
                                start=(kt == 0),
                                stop=(kt == 1),
                            )
                        o32 = op_.tile([128, SEG], DTF, tag="o32", name="o32")
                        nc.vector.tensor_copy(o32[:], ps[:])
                        nc.sync.dma_start(ot[s, b, h * 128 : (h + 1) * 128, :], o32[:])

                nc.gpsimd.collective_compute(
                    "AllReduce", mybir.AluOpType.add, replica_groups=RG,
                    ins=[starin[s].opt()], outs=[starout[s].opt()],
                )
                if s + 1 < nseg:
                    for b in range(B):
                        for kt in range(KT1):
                            t32 = tp.tile([128, ST], DTF, tag="st32", name="st32")
                            nc.sync.dma_start(
                                t32[:],
                                starout[s][b * D + kt * 128 : b * D + (kt + 1) * 128, :],
                            )
                            nc.vector.tensor_copy(fullS[b, kt][:], t32[:])
                            nc.vector.tensor_copy(fullE[b, kt][:], t32[:])

    nc.compile()
    return nc


# ======================================================================
# host-side weight preparation
# ======================================================================
def _prep(x, params):
    lp0, lp1 = params["layers"]
    scale_q = 1.0 / np.sqrt(np.float32(DK))

    def regs(lp, nm):
        return [np.asarray(lp[nm + sfx], dtype=np.float32) for sfx in ("_s", "", "_e")]

    winv = [np.asarray(lp0[nm], dtype=np.float32) for nm in ("Winv_b", "Winv", "Winv_e")]

    # per-head weight blocks, laid out exactly as their SBUF tiles
    def pack_small(w_g, dout):
        # w_g: (256, dout) -> (128, KT1*dout): [p, kt*dout+c] = w[kt*128+p, c]
        return np.concatenate([w_g[kt * 128 : (kt + 1) * 128] for kt in range(KT1)], axis=1)

    def pack_big(w_g, dout):
        # w_g: (2048, dout) -> (128, KT*dout)
        return np.concatenate([w_g[kt * 128 : (kt + 1) * 128] for kt in range(KT)], axis=1)

    per_core = [dict() for _ in range(NC)]

    # qkv1: head-summed weights (input is head-broadcast)
    for nm, out_nm, perm, sc in (("Wq", "w1q", True, scale_q), ("Wk", "w1k", True, 1.0),
                                 ("Wv", "w1v", False, 1.0)):
        rr = regs(lp0, nm)
        for g in range(NC):
            blocks = []
            for r in range(3):
                w = rr[r].sum(axis=0)[g] * sc          # (256, dout)
                if perm:
                    w = w[:, _PERM]
                blocks.append(pack_small(w, w.shape[1]))
            per_core[g][out_nm] = np.stack(blocks).astype(np.float16)

    # fused proj_inv + qkv2
    for nm, out_nm, perm, sc in (("Wq", "w2q", True, scale_q), ("Wk", "w2k", True, 1.0),
                                 ("Wv", "w2v", False, 1.0)):
        rr = regs(lp0, nm)
        for r in range(3):
            # (h, f, d, o) = winv[h,g,d,e] @ wq[g,f,e,o]
            wf = np.einsum("hgde,gfeo->hfdo", winv[r], rr[r], optimize=True) * sc
            for g in range(NC):
                w = wf[:, g].reshape(H * D, -1)        # (2048, dout), h-major
                if perm:
                    w = w[:, _PERM]
                per_core[g].setdefault(out_nm, [None] * 3)[r] = pack_big(w, w.shape[1])
        for g in range(NC):
            per_core[g][out_nm] = np.stack(per_core[g][out_nm]).astype(np.float16)

    # layer-1 qkv
    for nm, out_nm, perm, sc in (("Wq", "w3q", True, scale_q), ("Wk", "w3k", True, 1.0),
                                 ("Wv", "w3v", False, 1.0)):
        rr = regs(lp1, nm)
        for g in range(NC):
            blocks = []
            for r in range(3):
                w = rr[r].transpose(1, 0, 2, 3)[g].reshape(H * D, -1) * sc
                if perm:
                    w = w[:, _PERM]
                blocks.append(pack_big(w, w.shape[1]))
            per_core[g][out_nm] = np.stack(blocks).astype(np.float16)

    wout = np.asarray(params["W_out"], dtype=np.float32) / H
    wst = np.asarray(params["W_state"], dtype=np.float32) * (0.5 / H)
    wout_p = np.stack([wout[k * 128 : (k + 1) * 128] for k in range(2)]).astype(np.float16)
    wst_p = np.stack([wst[k * 128 : (k + 1) * 128] for k in range(2)]).astype(np.float16)

    # rope tables (permuted order, duplicated halves)
    pos = (np.arange(NSEG)[:, None] * SEG + np.arange(L)[None, :]).astype(np.float64)
    inv = 1.0 / (10000.0 ** (np.arange(0, DK, 2) / DK))
    ang = pos[:, :, None] * inv[None, None, :]          # (NSEG, L, 64)
    cos = np.cos(ang).transpose(0, 2, 1)                # (NSEG, 64, L)
    sin = np.sin(ang).transpose(0, 2, 1)
    ropeC = np.concatenate([cos, cos], axis=1).astype(np.float16)
    ropeS = np.concatenate([sin, sin], axis=1).astype(np.float16)

    mask = (np.arange(128)[:, None] <= np.arange(128)[None, :]).astype(np.float16)

    xT = np.ascontiguousarray(np.asarray(x, dtype=np.float32).transpose(0, 2, 1)).astype(np.float16)
    istT = np.ascontiguousarray(
        np.asarray(params["init_state"], dtype=np.float32)[0].T
    ).astype(np.float16)

    in_maps = []
    for g in range(NC):
        m = dict(per_core[g])
        m["xT"] = xT
        m["istT"] = istT
        m["ropeC"] = ropeC
        m["ropeS"] = ropeS
        m["wout"] = wout_p
        m["wst"] = wst_p
        m["mask01"] = mask
        in_maps.append(m)
    return in_maps


def kernel(x, params):
    if "nc" not in _compiled:
        _compiled["nc"] = _build()
    nc = _compiled["nc"]
    in_maps = _prep(x, params)
    res = run_bass_kernel_spmd(nc, in_maps, list(range(NC)))
    out = np.zeros((B, S, D), dtype=np.float32)
    for c in range(NC):
        o = res.results[c]["ot"]                        # (NSEG, B, D, SEG) fp32
        for s in range(NSEG):
            for b in range(B):
                out[b, s * SEG : (s + 1) * SEG] += o[s, b].T
    return out


# revision 14
# speedup vs baseline: 1.0403x; 1.0031x over previous
"""Trainium2 Bass kernel for nn_ReMMTAS (sparse stateful causal attention).

Strategy: 8-way tensor parallelism over the head axis (1 head per NeuronCore),
batch (B=2) kept on-core, segments sequential (state recurrence).

Math restructurings (all host-side, exact):
  - First attention's qkv: the input is head-broadcast, so StackedLinear
    collapses to a (256 -> dout) matmul with head-summed weights.
  - proj_inv + second qkv fused into one weight (Winv @ Wq2 per region),
    removing one head-mixing round-trip and one collective per segment.
  - RoPE feature pairs permuted (even/odd -> half-split) so the rotation is
    two partition-half blocks; permutation folded into q/k weight columns
    (scores are permutation-invariant).
  - 1/sqrt(dk) folded into q weights; W_out/H and W_state*0.5/H folded so
    per-head partial outputs simply sum across cores (host does the final sum
    for out_seg; a small fp32 AllReduce carries the recurrent state).

On-device dataflow per segment (per core, head g, per batch):
  fullT(256,768) -> qkv1 -> attn -> a1T(256,768) --AllGather--> (2048,768)
  -> fused qkv2 -> attn -> a2T --AllGather--> qkv3 -> attn -> a3T
  -> partial out_seg (fp32, host-summed) + partial state (fp32 AllReduce).

Everything on PE runs in fp16 (fp32 PSUM accumulation); softmax runs
unnormalized without max-subtraction (scores are small by construction),
with the denominator from an all-ones stationary matmul and applied via
DVE reciprocal+multiply after the PV matmul.
"""
import numpy as np

import concourse.bacc as bacc
import concourse.mybir as mybir
from concourse import tile
from concourse.bass_utils import run_bass_kernel_spmd

# ---- problem geometry (hardcoded per spec) ----
B, S, D = 2, 2048, 256
H = 8
SEG, ST = 512, 128
L = ST + SEG + ST          # 768
NSEG = S // SEG            # 4
DK, DV = 128, 256
NC = 8
KT1 = D // 128             # 2   (qkv1 contraction tiles)
KT = (H * D) // 128        # 16  (mixing contraction tiles)
LT = L // 128              # 6
REGIONS = ((0, ST), (ST, ST + SEG), (ST + SEG, L))

DT = mybir.dt.float16
DTF = mybir.dt.float32

_PERM = np.concatenate([np.arange(0, DK, 2), np.arange(1, DK, 2)])

_compiled = {}


# ======================================================================
# device program
# ======================================================================
def _build(nseg=NSEG):
    nc = bacc.Bacc("TRN2", target_bir_lowering=False, debug=False, num_devices=NC)

    xT = nc.dram_tensor("xT", [B, D, S], DT, kind="ExternalInput")
    istT = nc.dram_tensor("istT", [D, ST], DT, kind="ExternalInput")
    ropeC = nc.dram_tensor("ropeC", [NSEG, DK, L], DT, kind="ExternalInput")
    ropeS = nc.dram_tensor("ropeS", [NSEG, DK, L], DT, kind="ExternalInput")
    w1q = nc.dram_tensor("w1q", [3, 128, KT1 * 128], DT, kind="ExternalInput")
    w1k = nc.dram_tensor("w1k", [3, 128, KT1 * 128], DT, kind="ExternalInput")
    w1v = nc.dram_tensor("w1v", [3, 128, KT1 * 256], DT, kind="ExternalInput")
    w2q = nc.dram_tensor("w2q", [3, 128, KT * 128], DT, kind="ExternalInput")
    w2k = nc.dram_tensor("w2k", [3, 128, KT * 128], DT, kind="ExternalInput")
    w2v = nc.dram_tensor("w2v", [3, 128, KT * 256], DT, kind="ExternalInput")
    w3q = nc.dram_tensor("w3q", [3, 128, KT * 128], DT, kind="ExternalInput")
    w3k = nc.dram_tensor("w3k", [3, 128, KT * 128], DT, kind="ExternalInput")
    w3v = nc.dram_tensor("w3v", [3, 128, KT * 256], DT, kind="ExternalInput")
    wout = nc.dram_tensor("wout", [2, 128, 256], DT, kind="ExternalInput")
    wst = nc.dram_tensor("wst", [2, 128, 256], DT, kind="ExternalInput")
    mask01 = nc.dram_tensor("mask01", [128, 128], DT, kind="ExternalInput")

    ot = nc.dram_tensor("ot", [NSEG, B, D, SEG], DTF, kind="ExternalOutput")

    RG = [list(range(NC))]

    with tile.TileContext(nc) as tc:
        with (
            tc.tile_pool(name="dram", bufs=1, space="DRAM") as dp,
            tc.tile_pool(name="consts", bufs=1) as cp,
            tc.tile_pool(name="vw", bufs=3) as vwp,          # streamed v-weights
            tc.tile_pool(name="rope", bufs=2) as rp,
            tc.tile_pool(name="aall", bufs=18) as ap,        # gathered activations
            tc.tile_pool(name="work", bufs=4) as wp,         # qT/kT
            tc.tile_pool(name="vsb", bufs=12) as vp,
            tc.tile_pool(name="expp", bufs=12) as ep,
            tc.tile_pool(name="atp", bufs=4) as atp,
            tc.tile_pool(name="tmp", bufs=4) as tp,          # fp32 rope tmps
            tc.tile_pool(name="outp", bufs=2) as op_,
            tc.tile_pool(name="ps_qk", bufs=2, space="PSUM") as ps_qk,
            tc.tile_pool(name="ps_sv", bufs=2, space="PSUM") as ps_sv,
            tc.tile_pool(name="ps_acc", bufs=2, space="PSUM") as ps_acc,
        ):
            agin, agout, starin, starout = {}, {}, {}, {}
            for s in range(nseg):
                for m in (1, 2):
                    for b in range(B):
                        agin[s, m, b] = dp.tile([D, L], DT, tag=f"agin_{s}_{m}_{b}", name=f"agin_{s}_{m}_{b}")
                        agout[s, m, b] = dp.tile(
                            [H * D, L], DT, tag=f"agout_{s}_{m}_{b}",
                            addr_space="Shared", name=f"agout_{s}_{m}_{b}"
                        )
                starin[s] = dp.tile([B * D, ST], DTF, tag=f"starin_{s}", name=f"starin_{s}")
                starout[s] = dp.tile(
                    [B * D, ST], DTF, tag=f"starout_{s}",
                    addr_space="Shared", name=f"starout_{s}"
                )
            # ---------- resident constants ----------
            def load_const(name, dram, shape, sl=None):
                t = cp.tile(shape, DT, tag=name, name=name)
                nc.sync.dma_start(t[:], dram if sl is None else dram[sl])
                return t

            w1q_t = [load_const(f"w1q{r}", w1q[r], [128, KT1 * 128]) for r in range(3)]
            w1k_t = [load_const(f"w1k{r}", w1k[r], [128, KT1 * 128]) for r in range(3)]
            w1v_t = [load_const(f"w1v{r}", w1v[r], [128, KT1 * 256]) for r in range(3)]
            w2q_t = [load_const(f"w2q{r}", w2q[r], [128, KT * 128]) for r in range(3)]
            w2k_t = [load_const(f"w2k{r}", w2k[r], [128, KT * 128]) for r in range(3)]
            w3q_t = [load_const(f"w3q{r}", w3q[r], [128, KT * 128]) for r in range(3)]
            w3k_t = [load_const(f"w3k{r}", w3k[r], [128, KT * 128]) for r in range(3)]
            wout_t = [load_const(f"wout{k}", wout[k], [128, 256]) for k in range(2)]
            wst_t = [load_const(f"wst{k}", wst[k], [128, 256]) for k in range(2)]
            mask_t = load_const("mask", mask01[:], [128, 128])
            ones_t = cp.tile([128, 128], DT, tag="ones", name="ones_t")
            nc.vector.memset(ones_t[:], 1.0)

            # per-region full tiles; S/E hold the recurrent state directly.
            # M is double-buffered by segment parity and prefetched a segment
            # early so its DMA never queues behind the state AllReduce.
            fullS, fullM, fullE = {}, {}, {}
            for b in range(B):
                for kt in range(KT1):
                    fullS[b, kt] = cp.tile([128, ST], DT, tag=f"fs{b}_{kt}", name=f"fs{b}_{kt}")
                    fullE[b, kt] = cp.tile([128, ST], DT, tag=f"fe{b}_{kt}", name=f"fe{b}_{kt}")
                    for p in range(2):
                        fullM[b, kt, p] = cp.tile(
                            [128, SEG], DT, tag=f"fm{b}_{kt}{p}", name=f"fm{b}_{kt}{p}"
                        )
                    nc.sync.dma_start(fullS[b, kt][:], istT[kt * 128 : (kt + 1) * 128, :])
                    nc.sync.dma_start(fullE[b, kt][:], istT[kt * 128 : (kt + 1) * 128, :])

            def load_fullM(s):
                for b in range(B):
                    for kt in range(KT1):
                        nc.sync.dma_start(
                            fullM[b, kt, s % 2][:],
                            xT[b, kt * 128 : (kt + 1) * 128, s * SEG : (s + 1) * SEG],
                        )

            # ---------- helpers ----------
            def chunks(lo, hi, step=512):
                # matmul outputs must not cross a PSUM bank boundary (512 fp32)
                out = []
                c = lo
                while c < hi:
                    bank_end = (c // 512 + 1) * 512
                    out.append((c, min(c + step, hi, bank_end)))
                    c = out[-1][1]
                return out

            def proj_qk(src, w_t, nkt, ropeC_t, ropeS_t, label):
                """q/k projection + rope: returns (128, L) fp16 tile.
                src: either flat list of kt tiles (128, L), or (srcS, srcM, srcE)
                per-region kt tile lists."""
                per_region = isinstance(src, tuple)
                ps = ps_qk.tile([DK, L], DTF, tag="qk", name="ps_qk_t")
                for r in (1, 0, 2):
                    lo, hi = REGIONS[r]
                    for (c0, c1) in chunks(lo, hi):
                        for kt in range(nkt):
                            mv = (src[r][kt][:, c0 - lo : c1 - lo] if per_region
                                  else src[kt][:, c0:c1])
                            nc.tensor.matmul(
                                ps[:, c0:c1],
                                w_t[r][:, kt * 128 : (kt + 1) * 128],
                                mv,
                                start=(kt == 0),
                                stop=(kt == nkt - 1),
                            )
                # rope: out[0:64] = ps[0:64]*C - ps[64:]*S ; out[64:] = ps[64:]*C + ps[0:64]*S
                t = tp.tile([DK, L], DTF, tag="ropet", name="ropet")
                c = tp.tile([DK, L], DTF, tag="ropec", name="ropec")
                outT = wp.tile([DK, L], DT, tag=label, name=label)
                nc.vector.tensor_mul(t[0:64, :], ps[64:128, :], ropeS_t[0:64, :])
                nc.vector.tensor_mul(t[64:128, :], ps[0:64, :], ropeS_t[64:128, :])
                nc.vector.tensor_mul(c[:], ps[:], ropeC_t[:])
                nc.vector.tensor_sub(outT[0:64, :], c[0:64, :], t[0:64, :])
                nc.vector.tensor_add(outT[64:128, :], c[64:128, :], t[64:128, :])
                return outT

            def proj_v(src, w_t, nkt):
                """v projection in token-partition layout: 6 tiles (128, 256) fp16."""
                per_region = isinstance(src, tuple)
                v_tiles = [None] * LT
                for lt in (1, 2, 3, 4, 0, 5):
                    r = 0 if lt == 0 else (2 if lt == LT - 1 else 1)
                    ps = ps_sv.tile([128, DV], DTF, tag="sv", name="ps_v_t")
                    for kt in range(nkt):
                        if per_region:
                            off = lt * 128 - REGIONS[r][0]
                            st_op = src[r][kt][:, off : off + 128]
                        else:
                            st_op = src[kt][:, lt * 128 : (lt + 1) * 128]
                        nc.tensor.matmul(
                            ps[:],
                            st_op,
                            w_t[r][:, kt * 256 : (kt + 1) * 256],
                            start=(kt == 0),
                            stop=(kt == nkt - 1),
                        )
                    vt = vp.tile([128, DV], DT, tag="vsb", name="vsb_t")
                    nc.vector.tensor_copy(vt[:], ps[:])
                    v_tiles[lt] = vt
                return v_tiles

            def attention(qT, kT, v_tiles):
                """causal attention; returns a^T as 2 tiles (128, L) fp16."""
                # scoresT blocks (j-partition, i-free), chunked at the 384 split
                expT = []
                for jt in range(LT):
                    et = ep.tile([128, L], DT, tag="exp", name="exp_t")
                    ccs = []
                    if jt * 128 < 384:
                        ccs.append((jt * 128, 384))
                    ccs.append((max(jt * 128, 384), L))
                    for (c0, c1) in ccs:
                        ps = ps_sv.tile([128, 384], DTF, tag="sv", name="ps_s_t")
                        nc.tensor.matmul(
                            ps[:, 0 : c1 - c0],
                            kT[:, jt * 128 : (jt + 1) * 128],
                            qT[:, c0:c1],
                            start=True,
                            stop=True,
                        )
                        nc.scalar.activation(
                            et[:, c0:c1], ps[:, 0 : c1 - c0],
                            mybir.ActivationFunctionType.Exp,
                        )
                    # causal mask on the diagonal block
                    nc.vector.tensor_mul(
                        et[:, jt * 128 : (jt + 1) * 128],
                        et[:, jt * 128 : (jt + 1) * 128],
                        mask_t[:],
                    )
                    expT.append(et)

                aT = [atp.tile([128, L], DT, tag="aT", name="aT_t") for _ in range(2)]
                for (i0, i1) in ((0, 384), (384, L)):
                    acc0 = ps_acc.tile([128, 384], DTF, tag="acc", name="acc0")
                    acc1 = ps_acc.tile([128, 384], DTF, tag="acc", name="acc1")
                    accs = ps_sv.tile([128, 384], DTF, tag="sv", name="accs")
                    jts = [jt for jt in range(LT) if jt * 128 < i1]
                    for n, jt in enumerate(jts):
                        c0 = max(jt * 128, i0)
                        first, last = (n == 0), (n == len(jts) - 1)
                        src = expT[jt][:, c0:i1]
                        dst = slice(c0 - i0, i1 - i0)
                        nc.tensor.matmul(acc0[:, dst], v_tiles[jt][:, 0:128], src,
                                         start=first, stop=last)
                        nc.tensor.matmul(acc1[:, dst], v_tiles[jt][:, 128:256], src,
                                         start=first, stop=last)
                        nc.tensor.matmul(accs[:, dst], ones_t[:], src,
                                         start=first, stop=last)
                    rec = tp.tile([128, 384], DTF, tag="rec", name="rec")
                    nc.vector.reciprocal_approx_fast(rec[:], accs[:])
                    nc.vector.tensor_mul(aT[0][:, i0:i1], acc0[:], rec[:])
                    nc.vector.tensor_mul(aT[1][:, i0:i1], acc1[:], rec[:])
                return aT

            # ---------- main sequence ----------
            for s in range(nseg):
                ropeC_t = rp.tile([DK, L], DT, tag="ropeC", name="ropeC_t")
                ropeS_t = rp.tile([DK, L], DT, tag="ropeS", name="ropeS_t")
                nc.sync.dma_start(ropeC_t[:], ropeC[s])
                nc.sync.dma_start(ropeS_t[:], ropeS[s])
                if s == 0:
                    load_fullM(0)
                if s + 1 < nseg:
                    load_fullM(s + 1)
                w2v_t = [vwp.tile([128, KT * 256], DT, tag="vw", name="w2v_t") for _ in range(3)]
                for r in range(3):
                    nc.sync.dma_start(w2v_t[r][:], w2v[r])

                a1T = {}
                for b in range(B):
                    src = (
                        [fullS[b, 0], fullS[b, 1]],
                        [fullM[b, 0, s % 2], fullM[b, 1, s % 2]],
                        [fullE[b, 0], fullE[b, 1]],
                    )
                    vt = proj_v(src, w1v_t, KT1)
                    qT = proj_qk(src, w1q_t, KT1, ropeC_t, ropeS_t, "qT")
                    kT = proj_qk(src, w1k_t, KT1, ropeC_t, ropeS_t, "kT")
                    a1T[b] = attention(qT, kT, vt)
                    for i in range(2):
                        for (c0, c1) in ((0, 384), (384, L)):
                            nc.sync.dma_start(
                                agin[s, 1, b][i * 128 : (i + 1) * 128, c0:c1],
                                a1T[b][i][:, c0:c1],
                            )
                    nc.gpsimd.collective_compute(
                        "AllGather", mybir.AluOpType.bypass, replica_groups=RG,
                        ins=[agin[s, 1, b].opt()], outs=[agout[s, 1, b].opt()],
                    )

                w3v_t = [vwp.tile([128, KT * 256], DT, tag="vw", name="w3v_t") for _ in range(3)]
                for r in range(3):
                    nc.sync.dma_start(w3v_t[r][:], w3v[r])

                a2T = {}
                for b in range(B):
                    aall = [ap.tile([128, L], DT, tag="aall", name="aall_t") for _ in range(KT)]
                    for kt in range(KT):
                        nc.sync.dma_start(
                            aall[kt][:], agout[s, 1, b][kt * 128 : (kt + 1) * 128, :]
                        )
                    vt = proj_v(aall, w2v_t, KT)
                    qT = proj_qk(aall, w2q_t, KT, ropeC_t, ropeS_t, "qT")
                    kT = proj_qk(aall, w2k_t, KT, ropeC_t, ropeS_t, "kT")
                    a2T[b] = attention(qT, kT, vt)
                    for i in range(2):
                        for (c0, c1) in ((0, 384), (384, L)):
                            nc.sync.dma_start(
                                agin[s, 2, b][i * 128 : (i + 1) * 128, c0:c1],
                                a2T[b][i][:, c0:c1],
                            )
                    nc.gpsimd.collective_compute(
                        "AllGather", mybir.AluOpType.bypass, replica_groups=RG,
                        ins=[agin[s, 2, b].opt()], outs=[agout[s, 2, b].opt()],
                    )

                for b in range(B):
                    aall = [ap.tile([128, L], DT, tag="aall", name="aall_t") for _ in range(KT)]
                    for kt in range(KT):
                        nc.sync.dma_start(
                            aall[kt][:], agout[s, 2, b][kt * 128 : (kt + 1) * 128, :]
                        )
                    vt = proj_v(aall, w3v_t, KT)
                    qT = proj_qk(aall, w3q_t, KT, ropeC_t, ropeS_t, "qT")
                    kT = proj_qk(aall, w3k_t, KT, ropeC_t, ropeS_t, "kT")
                    a3T = attention(qT, kT, vt)

                    # state partial first (feeds the AllReduce on the critical path)
                    stA = [] if s + 1 < nseg else None
                    if stA is not None:
                        for kt in range(2):
                            t = op_.tile([128, ST], DT, tag="stA", name="stA_t")
                            nc.vector.tensor_add(
                                t[:], a3T[kt][:, 0:ST], a3T[kt][:, ST + SEG : L]
                            )
                            stA.append(t)
                        for h in range(2):
                            ps = ps_sv.tile([128, ST], DTF, tag="sv", name="ps_st")
                            for kt in range(2):
                                nc.tensor.matmul(
                                    ps[:],
                                    wst_t[kt][:, h * 128 : (h + 1) * 128],
                                    stA[kt][:],
                                    start=(kt == 0),
                                    stop=(kt == 1),
                                )
                            s32 = op_.tile([128, ST], DTF, tag="s32", name="s32")
                            nc.vector.tensor_copy(s32[:], ps[:])
                            nc.sync.dma_start(
                                starin[s][b * D + h * 128 : b * D + (h + 1) * 128, :],
                                s32[:],
                            )

                    # out partial: oT[h] = wout[:,h].T @ a3T[:, ST:ST+SEG]
                    for h in range(2):
                        ps = ps_qk.tile([128, SEG], DTF, tag="qk", name="ps_o")
                        for kt in range(2):
                            nc.tensor.matmul(
                                ps[:],
                                wout_t[kt][:, h * 128 : (h + 1) * 128],
                                a3T[kt][:, ST : ST + SEG],
                                start=(kt == 0),
                                stop=(kt == 1),
                            )
                        o32 = op_.tile([128, SEG], DTF, tag="o32", name="o32")
                        nc.vector.tensor_copy(o32[:], ps[:])
                        nc.sync.dma_start(ot[s, b, h * 128 : (h + 1) * 128, :], o32[:])

                if s + 1 < nseg:
                    nc.gpsimd.collective_compute(
                        "AllReduce", mybir.AluOpType.add, replica_groups=RG,
                        ins=[starin[s].opt()], outs=[starout[s].opt()],
                    )
                    for b in range(B):
                        for kt in range(KT1):
                            t32 = tp.tile([128, ST], DTF, tag="st32", name="st32")
                            nc.sync.dma_start(
                                t32[:],
                                starout[s][b * D + kt * 128 : b * D + (kt + 1) * 128, :],
                            )
                            nc.vector.tensor_copy(fullS[b, kt][:], t32[:])
                            nc.vector.tensor_copy(fullE[b, kt][:], t32[:])

    nc.compile()
    return nc


# ======================================================================
# host-side weight preparation
# ======================================================================
def _prep(x, params):
    lp0, lp1 = params["layers"]
    scale_q = 1.0 / np.sqrt(np.float32(DK))

    def regs(lp, nm):
        return [np.asarray(lp[nm + sfx], dtype=np.float32) for sfx in ("_s", "", "_e")]

    winv = [np.asarray(lp0[nm], dtype=np.float32) for nm in ("Winv_b", "Winv", "Winv_e")]

    # per-head weight blocks, laid out exactly as their SBUF tiles
    def pack_small(w_g, dout):
        # w_g: (256, dout) -> (128, KT1*dout): [p, kt*dout+c] = w[kt*128+p, c]
        return np.concatenate([w_g[kt * 128 : (kt + 1) * 128] for kt in range(KT1)], axis=1)

    def pack_big(w_g, dout):
        # w_g: (2048, dout) -> (128, KT*dout)
        return np.concatenate([w_g[kt * 128 : (kt + 1) * 128] for kt in range(KT)], axis=1)

    per_core = [dict() for _ in range(NC)]

    # qkv1: head-summed weights (input is head-broadcast)
    for nm, out_nm, perm, sc in (("Wq", "w1q", True, scale_q), ("Wk", "w1k", True, 1.0),
                                 ("Wv", "w1v", False, 1.0)):
        rr = regs(lp0, nm)
        for g in range(NC):
            blocks = []
            for r in range(3):
                w = rr[r].sum(axis=0)[g] * sc          # (256, dout)
                if perm:
                    w = w[:, _PERM]
                blocks.append(pack_small(w, w.shape[1]))
            per_core[g][out_nm] = np.stack(blocks).astype(np.float16)

    # fused proj_inv + qkv2
    for nm, out_nm, perm, sc in (("Wq", "w2q", True, scale_q), ("Wk", "w2k", True, 1.0),
                                 ("Wv", "w2v", False, 1.0)):
        rr = regs(lp0, nm)
        for r in range(3):
            # (h, f, d, o) = winv[h,g,d,e] @ wq[g,f,e,o]
            wf = np.einsum("hgde,gfeo->hfdo", winv[r], rr[r], optimize=True) * sc
            for g in range(NC):
                w = wf[:, g].reshape(H * D, -1)        # (2048, dout), h-major
                if perm:
                    w = w[:, _PERM]
                per_core[g].setdefault(out_nm, [None] * 3)[r] = pack_big(w, w.shape[1])
        for g in range(NC):
            per_core[g][out_nm] = np.stack(per_core[g][out_nm]).astype(np.float16)

    # layer-1 qkv
    for nm, out_nm, perm, sc in (("Wq", "w3q", True, scale_q), ("Wk", "w3k", True, 1.0),
                                 ("Wv", "w3v", False, 1.0)):
        rr = regs(lp1, nm)
        for g in range(NC):
            blocks = []
            for r in range(3):
                w = rr[r].transpose(1, 0, 2, 3)[g].reshape(H * D, -1) * sc
                if perm:
                    w = w[:, _PERM]
                blocks.append(pack_big(w, w.shape[1]))
            per_core[g][out_nm] = np.stack(blocks).astype(np.float16)

    wout = np.asarray(params["W_out"], dtype=np.float32) / H
    wst = np.asarray(params["W_state"], dtype=np.float32) * (0.5 / H)
    wout_p = np.stack([wout[k * 128 : (k + 1) * 128] for k in range(2)]).astype(np.float16)
    wst_p = np.stack([wst[k * 128 : (k + 1) * 128] for k in range(2)]).astype(np.float16)

    # rope tables (permuted order, duplicated halves)
    pos = (np.arange(NSEG)[:, None] * SEG + np.arange(L)[None, :]).astype(np.float64)
    inv = 1.0 / (10000.0 ** (np.arange(0, DK, 2) / DK))
    ang = pos[:, :, None] * inv[None, None, :]          # (NSEG, L, 64)
    cos = np.cos(ang).transpose(0, 2, 1)                # (NSEG, 64, L)
    sin = np.sin(ang).transpose(0, 2, 1)
    ropeC = np.concatenate([cos, cos], axis=1).astype(np.float16)
    ropeS = np.concatenate([sin, sin], axis=1).astype(np.float16)

    mask = (np.arange(128)[:, None] <= np.arange(128)[None, :]).astype(np.float16)

    xT = np.ascontiguousarray(np.asarray(x, dtype=np.float32).transpose(0, 2, 1)).astype(np.float16)
    istT = np.ascontiguousarray(
        np.asarray(params["init_state"], dtype=np.float32)[0].T
    ).astype(np.float16)

    in_maps = []
    for g in range(NC):
        m = dict(per_core[g])
        m["xT"] = xT
        m["istT"] = istT
        m["ropeC"] = ropeC
        m["ropeS"] = ropeS
        m["wout"] = wout_p
        m["wst"] = wst_p
        m["mask01"] = mask
        in_maps.append(m)
    return in_maps


def kernel(x, params):
    if "nc" not in _compiled:
        _compiled["nc"] = _build()
    nc = _compiled["nc"]
    in_maps = _prep(x, params)
    res = run_bass_kernel_spmd(nc, in_maps, list(range(NC)))
    out = np.zeros((B, S, D), dtype=np.float32)
    for c in range(NC):
        o = res.results[c]["ot"]                        # (NSEG, B, D, SEG) fp32
        for s in range(NSEG):
            for b in range(B):
                out[b, s * SEG : (s + 1) * SEG] += o[s, b].T
    return out
